# revision 33
# baseline (speedup 1.0000x reference)
"""Trainium2 Bass kernel for full-embed-dim self-attention + residual LayerNorm.

Problem: B=4, S=2048, D=1024 fp32.
  q/k/v = x@w{q,k,v}+b; scores = q@k^T/sqrt(D); attn = softmax(scores)@v;
  out = LN(x + attn@wo + bo) * gamma + beta.

Sharding: 8 cores = 4 batches x 2 query-halves (1024 queries each). Each
core computes K/V projections only for its own 1024 keys, then a pair-wise
AllGather ([0,1],[2,3],...) exchanges the halves so each core attends over
the batch's full 2048-key sequence.

All four matmul stages run in fp8(e4m3) DoubleRow mode: operands are packed
as [128, 2, N] pair tiles (two 128-row contraction chunks side by side), so
each matmul contracts 256 elements/instruction at 2 fp8 MACs/cell/cycle
(~1.9x measured over bf16; LDWEIGHTS fully pipelines behind the previous
matmul at 512-wide moving operands). PSUM accumulation stays fp32. e4m3
quantization noise (~3.6%/element) decorrelates over the 1024-2048-long
contractions; measured end-to-end max rel err is ~8e-3 vs the 2e-2 gate.
NOTE: DoubleRow weight APs require the pair-dim byte stride to be a
multiple of 16 - a 1-byte-stride ones vector hangs the PE (hence the
[128, 2, 16] ones tile).

Phase order (hides the two collectives behind compute; the CC engine has
a fixed ~50us init floor, so partner data can never arrive early -- the
schedule keeps the PE busy with own-half work until it does):
  KT proj -> AG-KT -> V proj -> AG-V -> QT proj -> scores over own keys ->
  attnV over own keys (m 0..3) into bf16 partials while the gathers land ->
  scores over partner keys -> denominators -> attnV over partner keys
  (drain adds the bf16 partial) interleaved with out-proj by query half so
  the LayerNorm epilogue of half 0 overlaps half 1's matmuls.

Per-core dataflow (d-on-partitions "transposed" layout throughout):
  QT[d,q]   = wq^T @ xq^T (+bq)      (host ships x^T fp8 for the core's half)
  KT[d,k_own] = wk^T @ xq^T (+bk)    --> AllGather --> KT full
  V[k_own,d]  = x @ wv               --> AllGather --> V full
  ST[k,q]   = KT^T @ QT              (contracted over d in PSUM)
  PT[k,q]   = exp(ST/32 - ln16)      (the /16 keeps PT in [0,11] and AT in
              ~[-45,45], inside e4m3's +-240 range; it cancels in the
              normalization since the denominator uses the same PT)
  AT[d,q]   = V^T @ PT ; denom[q] = ones^T @ PT  (DoubleRow ones vector)
  O[q,e]    = AT^T @ wo, then O/denom[q] + xq_aug (host adds bo+bv@wo,
              ships x+bo_eff as bf16), LayerNorm with the elementwise pass
              alternating between DVE and ACT per query block. gamma==1 /
              beta==0 (always true for this problem) takes a fused
              single-pass path; a general two-pass path is kept otherwise.
"""

import numpy as np
import ml_dtypes

import concourse.bass as bass
import concourse.mybir as mybir
import concourse.tile as tile
from concourse import bacc

F32 = mybir.dt.float32
F8 = mybir.dt.float8e4
BF16 = mybir.dt.bfloat16
DR = mybir.MatmulPerfMode.DoubleRow

B, S, D = 4, 2048, 1024
Q = 1024          # queries (and own keys) per core
SCALE = 1.0 / 32.0
NLOG16 = -2.772588722239781   # -ln(16): PT = exp(s)/16
EPS = 1e-6
NP = 4            # d pair-chunks (256 each)
NDC = 8           # 128-wide chunks per 1024
RG = [[0, 1], [2, 3], [4, 5], [6, 7]]


def _bcast_ap(ap_1d, parts=128):
    """[N] dram AP -> [parts, N] AP with 0-stride partition dim."""
    return bass.AP(
        tensor=ap_1d.tensor, offset=ap_1d.offset, ap=[[0, parts]] + list(ap_1d.ap)
    )


def build_nc(trivial_gb: bool = True):
    nc = bacc.Bacc("TRN2", target_bir_lowering=False, debug=False, num_devices=8)

    xt8_d = nc.dram_tensor("xt8", [D, Q], F8, kind="ExternalInput")
    xq_d = nc.dram_tensor("xq", [Q, D], BF16, kind="ExternalInput")
    wq_d = nc.dram_tensor("wq", [D, D], F8, kind="ExternalInput")
    wk_d = nc.dram_tensor("wk", [D, D], F8, kind="ExternalInput")
    wv_d = nc.dram_tensor("wv", [D, D], F8, kind="ExternalInput")
    wo_d = nc.dram_tensor("wo", [D, D], F8, kind="ExternalInput")
    bqT_d = nc.dram_tensor("bqT", [128, NDC], F32, kind="ExternalInput")
    bkT_d = nc.dram_tensor("bkT", [128, NDC], F32, kind="ExternalInput")
    gamma_d = nc.dram_tensor("gamma", [D], F32, kind="ExternalInput")
    beta_d = nc.dram_tensor("beta", [D], F32, kind="ExternalInput")
    out_d = nc.dram_tensor("out", [Q, D], F32, kind="ExternalOutput")

    with tile.TileContext(nc) as tc:
        with (
            tc.tile_pool(name="small", bufs=1) as p_small,
            tc.tile_pool(name="dram", bufs=1, space="DRAM") as p_dram,
            tc.tile_pool(name="qtp", bufs=NP) as p_qt,
            tc.tile_pool(name="ktp", bufs=NP) as p_kt,
            tc.tile_pool(name="ktlp", bufs=NP) as p_ktl,
            tc.tile_pool(name="vp", bufs=NDC) as p_v,
            tc.tile_pool(name="ptp", bufs=NDC) as p_pt,
            tc.tile_pool(name="atp", bufs=NP) as p_at,
        ):
            kvin_kt = p_dram.tile([D, Q], F8, name="kvin_kt")
            kvout_kt = p_dram.tile([2 * D, Q], F8, name="kvout_kt")
            kvin_v = p_dram.tile([Q, D], F8, name="kvin_v")
            kvout_v = p_dram.tile([S, D], F8, name="kvout_v")

            # ---- constants / small tiles ----
            bqT = p_small.tile([128, NDC], F32)
            nc.gpsimd.dma_start(out=bqT[:, :], in_=bqT_d[:, :])
            bkT = p_small.tile([128, NDC], F32)
            nc.gpsimd.dma_start(out=bkT[:, :], in_=bkT_d[:, :])
            # [128, 2, 16] so the DoubleRow pair dim strides 16 B (HW requires
            # weight-AP step % 16 == 0); only column 0 is used as the ones vector.
            ones = p_small.tile([128, 2, 16], F8)
            nc.vector.memset(ones[:, :, :], 1.0)
            nl16 = p_small.tile([128, 1], F32)
            nc.vector.memset(nl16[:, :], NLOG16)
            eps_t = p_small.tile([128, 1], F32)
            nc.vector.memset(eps_t[:, :], EPS)
            recip = p_small.tile([128, 8], F32)

            pid = nc.sync.partition_id()
            partner_off = (1 - (pid % 2)) * Q   # partner's row base in gathered buffers
            poff512 = (1 - (pid % 2)) * 512     # same, within a half-gather block

            qtp = [p_qt.tile([128, 2, Q], F8, tag="qt", name=f"qtp{j}") for j in range(NP)]
            ktp = [p_kt.tile([128, 2, Q], F8, tag="kt", name=f"ktp{j}") for j in range(NP)]
            ptp = [p_pt.tile([128, 2, Q], F8, tag="pt", name=f"ptp{m}") for m in range(NDC)]

            with (
                tc.tile_pool(name="wp", bufs=3 * NP) as p_w,
                tc.tile_pool(name="xp", bufs=NP) as p_x,
            ):
                wkp = [p_w.tile([128, 2, D], F8, tag="w", name=f"wkp{j}") for j in range(NP)]
                wqp = [p_w.tile([128, 2, D], F8, tag="w", name=f"wqp{j}") for j in range(NP)]
                wvp = [p_w.tile([128, 2, D], F8, tag="w", name=f"wvp{j}") for j in range(NP)]
                xp = [p_x.tile([128, 2, Q], F8, tag="x", name=f"xp{j}") for j in range(NP)]
                for j in range(NP):
                    for i in range(2):
                        r = 256 * j + 128 * i
                        xe = nc.sync if i == 0 else nc.scalar
                        we = nc.gpsimd if i == 0 else nc.scalar
                        xe.dma_start(out=xp[j][:, i, :], in_=xt8_d[r:r + 128, :])
                        we.dma_start(out=wkp[j][:, i, :], in_=wk_d[r:r + 128, :])
                for j in range(NP):
                    for i in range(2):
                        r = 256 * j + 128 * i
                        nc.sync.dma_start(out=wqp[j][:, i, :], in_=wq_d[r:r + 128, :])
                        nc.gpsimd.dma_start(out=wvp[j][:, i, :], in_=wv_d[r:r + 128, :])

                with tc.tile_pool(name="psp", bufs=6, space="PSUM") as p_ps_proj:
                    # ---- KT_own[d, k_own] (+bk) -> ktp pair tiles + DRAM bounce ----
                    for do in range(NDC):
                        pss = [p_ps_proj.tile([128, 512], F32, tag="ps", name=f"pskt{do}_{h}") for h in range(2)]
                        for j in range(NP):
                            for kh in range(2):
                                nc.tensor.matmul(
                                    pss[kh][:, :],
                                    wkp[j][:, :, 128 * do:128 * (do + 1)],
                                    xp[j][:, :, 512 * kh:512 * (kh + 1)],
                                    start=(j == 0), stop=(j == NP - 1), perf_mode=DR,
                                )
                        for kh in range(2):
                            nc.vector.tensor_scalar(
                                out=ktp[do // 2][:, do % 2, 512 * kh:512 * (kh + 1)],
                                in0=pss[kh][:, :],
                                scalar1=bkT[:, do:do + 1], scalar2=None,
                                op0=mybir.AluOpType.add,
                            )
                    # Exchange KT in two d-halves. Stage ALL kvin stores first:
                    # putting the half-B stores after the ktlp-A loads would
                    # head-of-line-block them on the sync queue behind the AG-A
                    # completion wait, serializing half B behind half A.
                    ktlp = [p_ktl.tile([128, 2, Q], F8, tag="ktl", name=f"ktlp{j}") for j in range(NP)]
                    for do in range(NDC):
                        nc.sync.dma_start(
                            out=kvin_kt[128 * do:128 * (do + 1), :],
                            in_=ktp[do // 2][:, do % 2, :],
                        )
                    nc.gpsimd.collective_compute(
                        "AllGather", mybir.AluOpType.bypass, replica_groups=RG,
                        ins=[kvin_kt[:, :].opt()], outs=[kvout_kt[:, :].opt()],
                    )
                    # partner KT via runtime-parity offset
                    for j in range(NP):
                        for i in range(2):
                            nc.sync.dma_start(
                                out=ktlp[j][:, i, :],
                                in_=kvout_kt[bass.ds(partner_off + 256 * j + 128 * i, 128), :],
                            )

                    # ---- V_own[k_own, e] -> vp pair tiles (m 0..3) + DRAM bounce ----
                    vp = [p_v.tile([128, 2, D], F8, tag="v", name=f"vp{m}") for m in range(NDC)]
                    for kl in range(NDC):
                        pss = [p_ps_proj.tile([128, 512], F32, tag="ps", name=f"psv{kl}_{h}") for h in range(2)]
                        for j in range(NP):
                            for dh in range(2):
                                nc.tensor.matmul(
                                    pss[dh][:, :],
                                    xp[j][:, :, 128 * kl:128 * (kl + 1)],
                                    wvp[j][:, :, 512 * dh:512 * (dh + 1)],
                                    start=(j == 0), stop=(j == NP - 1), perf_mode=DR,
                                )
                        for dh in range(2):
                            nc.scalar.activation(
                                out=vp[kl // 2][:, kl % 2, 512 * dh:512 * (dh + 1)],
                                in_=pss[dh][:, :],
                                func=mybir.ActivationFunctionType.Copy,
                                bias=0.0, scale=1.0,
                            )
                    for kl in range(NDC):
                        nc.gpsimd.dma_start(
                            out=kvin_v[128 * kl:128 * (kl + 1), :],
                            in_=vp[kl // 2][:, kl % 2, :],
                        )
                    nc.gpsimd.collective_compute(
                        "AllGather", mybir.AluOpType.bypass, replica_groups=RG,
                        ins=[kvin_v[:, :].opt()], outs=[kvout_v[:, :].opt()],
                    )
                    # partner V (local key chunks 8..15)
                    for m in range(4, NDC):
                        for i in range(2):
                            nc.sync.dma_start(
                                out=vp[m][:, i, :],
                                in_=kvout_v[bass.ds(partner_off + 256 * (m - 4) + 128 * i, 128), :],
                            )

                    # ---- QT[d,q] (+bq) ----
                    for do in range(NDC):
                        pss = [p_ps_proj.tile([128, 512], F32, tag="ps", name=f"psqt{do}_{h}") for h in range(2)]
                        for j in range(NP):
                            for qh in range(2):
                                nc.tensor.matmul(
                                    pss[qh][:, :],
                                    wqp[j][:, :, 128 * do:128 * (do + 1)],
                                    xp[j][:, :, 512 * qh:512 * (qh + 1)],
                                    start=(j == 0), stop=(j == NP - 1), perf_mode=DR,
                                )
                        for qh in range(2):
                            nc.vector.tensor_scalar(
                                out=qtp[do // 2][:, do % 2, 512 * qh:512 * (qh + 1)],
                                in0=pss[qh][:, :],
                                scalar1=bqT[:, do:do + 1], scalar2=None,
                                op0=mybir.AluOpType.add,
                            )


            with (
                tc.tile_pool(name="ps", bufs=6, space="PSUM") as p_ps,
                tc.tile_pool(name="ps1", bufs=2, space="PSUM") as p_ps1,
            ):
                # bf16 partials for the own-key half of attnV, accumulated while
                # the KT/V gathers are still in flight
                ato = [p_at.tile([128, 2, Q], BF16, tag="ato", name=f"ato{j}") for j in range(NP)]

                # ---- ST -> exp -> PT; local key order: kc 0..7 own, 8..15 partner ----
                for kc in list(range(8)) + ["attnv_own"] + list(range(8, 16)):
                    if kc == "attnv_own":
                        # attnV over OWN keys (m 0..3) -> bf16 partials; fills
                        # the PE while AG-KT/AG-V complete
                        for qh in range(2):
                            for dc in range(NDC):
                                ps = p_ps.tile([128, 512], F32, tag="ps", name=f"psao{qh}_{dc}")
                                for m in range(4):
                                    nc.tensor.matmul(
                                        ps[:, :],
                                        vp[m][:, :, 128 * dc:128 * (dc + 1)],
                                        ptp[m][:, :, 512 * qh:512 * (qh + 1)],
                                        start=(m == 0), stop=(m == 3), perf_mode=DR,
                                    )
                                nc.vector.tensor_copy(
                                    ato[dc // 2][:, dc % 2, 512 * qh:512 * (qh + 1)], ps[:, :]
                                )
                        continue
                    kt_j = ktp if kc < NDC else ktlp
                    kcl = kc % NDC
                    pss = [p_ps.tile([128, 512], F32, tag="ps", name=f"psst{kc}_{h}") for h in range(2)]
                    for j in range(NP):
                        for qh in range(2):
                            nc.tensor.matmul(
                                pss[qh][:, :],
                                kt_j[j][:, :, 128 * kcl:128 * (kcl + 1)],
                                qtp[j][:, :, 512 * qh:512 * (qh + 1)],
                                start=(j == 0), stop=(j == NP - 1), perf_mode=DR,
                            )
                    for qh in range(2):
                        nc.scalar.activation(
                            out=ptp[kc // 2][:, kc % 2, 512 * qh:512 * (qh + 1)],
                            in_=pss[qh][:, :],
                            func=mybir.ActivationFunctionType.Exp,
                            bias=nl16[:, :], scale=SCALE,
                        )

                # ---- denominators: denom[q] = ones^T @ PT ----
                for qp in range(8):
                    ps1 = p_ps1.tile([128, 1], F32, tag="ps1", name=f"ps1_{qp}")
                    for m in range(NDC):
                        nc.tensor.matmul(
                            ps1[:, :],
                            ptp[m][:, :, 128 * qp:128 * (qp + 1)],
                            ones[:, :, 0:1],
                            start=(m == 0), stop=(m == NDC - 1), perf_mode=DR,
                        )
                    nc.vector.reciprocal(recip[:, qp:qp + 1], ps1[:, :])

                # ---- AT[d,q] = V^T @ PT by query-half, with the output
                # projection + LayerNorm for that half interleaved so the
                # elementwise epilogue overlaps the other half's matmuls ----
                atp = [p_at.tile([128, 2, Q], F8, tag="at", name=f"atp{j}") for j in range(NP)]
                with (
                    tc.tile_pool(name="wop", bufs=NP) as p_wo,
                    tc.tile_pool(name="xqp", bufs=4) as p_xq,
                    tc.tile_pool(name="vout", bufs=6) as p_vo,
                    tc.tile_pool(name="lnst", bufs=8) as p_ln,
                ):
                    wop = [p_wo.tile([128, 2, D], F8, tag="wo", name=f"wop{j}") for j in range(NP)]
                    for j in range(NP):
                        for i in range(2):
                            r = 256 * j + 128 * i
                            nc.scalar.dma_start(out=wop[j][:, i, :], in_=wo_d[r:r + 128, :])
                    gam = p_small.tile([128, D], F32)
                    bet = p_small.tile([128, D], F32)
                    if not trivial_gb:
                        nc.gpsimd.dma_start(out=gam[:, :], in_=_bcast_ap(gamma_d[:]))
                        nc.gpsimd.dma_start(out=bet[:, :], in_=_bcast_ap(beta_d[:]))

                    for qh in range(2):
                        for dc in range(NDC):
                            ps = p_ps.tile([128, 512], F32, tag="ps", name=f"psat{qh}_{dc}")
                            for m in range(4, NDC):
                                nc.tensor.matmul(
                                    ps[:, :],
                                    vp[m][:, :, 128 * dc:128 * (dc + 1)],
                                    ptp[m][:, :, 512 * qh:512 * (qh + 1)],
                                    start=(m == 4), stop=(m == NDC - 1), perf_mode=DR,
                                )
                            # AT = partner partial (psum) + own partial (bf16)
                            nc.vector.tensor_add(
                                atp[dc // 2][:, dc % 2, 512 * qh:512 * (qh + 1)],
                                ps[:, :],
                                ato[dc // 2][:, dc % 2, 512 * qh:512 * (qh + 1)],
                            )

                        for qp in range(4 * qh, 4 * qh + 4):
                            v = p_vo.tile([128, D], BF16, tag="v", name=f"v{qp}")
                            sqs = p_vo.tile([128, D], BF16, tag="sqs", name=f"sqs{qp}")
                            xqt_ = p_xq.tile([128, D], BF16, tag="xq", name=f"xqt{qp}")
                            nc.scalar.dma_start(
                                out=xqt_[:, :], in_=xq_d[128 * qp:128 * (qp + 1), :]
                            )
                            st = p_ln.tile([128, 8], F32, tag="st", name=f"st{qp}")
                            for eh in range(2):
                                ps = p_ps.tile([128, 512], F32, tag="ps", name=f"pso{qp}_{eh}")
                                for j in range(NP):
                                    nc.tensor.matmul(
                                        ps[:, :],
                                        atp[j][:, :, 128 * qp:128 * (qp + 1)],
                                        wop[j][:, :, 512 * eh:512 * (eh + 1)],
                                        start=(j == 0), stop=(j == NP - 1), perf_mode=DR,
                                    )
                                # v_half = O/denom + xq_aug; accum = row-sum
                                nc.vector.scalar_tensor_tensor(
                                    out=v[:, 512 * eh:512 * (eh + 1)], in0=ps[:, :],
                                    scalar=recip[:, qp:qp + 1],
                                    in1=xqt_[:, 512 * eh:512 * (eh + 1)],
                                    op0=mybir.AluOpType.mult, op1=mybir.AluOpType.add,
                                    accum_out=st[:, eh:eh + 1],
                                )
                            # E[v^2] via ACT Square + free accum; then mean/var/rstd
                            nc.scalar.activation(
                                out=sqs[:, :], in_=v[:, :],
                                func=mybir.ActivationFunctionType.Square,
                                accum_out=st[:, 2:3],
                            )
                            # mean = (s0+s1)/D ; var = sq/D - mean^2
                            nc.vector.tensor_scalar(
                                out=st[:, 0:1], in0=st[:, 0:1],
                                scalar1=st[:, 1:2], scalar2=1.0 / D,
                                op0=mybir.AluOpType.add, op1=mybir.AluOpType.mult,
                            )
                            nc.vector.tensor_mul(st[:, 1:2], st[:, 0:1], st[:, 0:1])
                            nc.vector.tensor_scalar(
                                out=st[:, 2:3], in0=st[:, 2:3],
                                scalar1=1.0 / D, scalar2=st[:, 1:2],
                                op0=mybir.AluOpType.mult, op1=mybir.AluOpType.subtract,
                            )
                            nc.scalar.activation(
                                out=st[:, 2:3], in_=st[:, 2:3],
                                func=mybir.ActivationFunctionType.Sqrt,
                                bias=eps_t[:, :],
                            )
                            nc.vector.reciprocal(st[:, 2:3], st[:, 2:3])       # rstd
                            vo = p_vo.tile([128, D], F32, tag="vo", name=f"vo{qp}")
                            if trivial_gb:
                                # gamma==1, beta==0: out = (v - mean)*rstd in one
                                # pass, alternating DVE / ACT per block.
                                if qp % 2 == 0:
                                    nc.vector.tensor_scalar(
                                        out=vo[:, :], in0=v[:, :],
                                        scalar1=st[:, 0:1], scalar2=st[:, 2:3],
                                        op0=mybir.AluOpType.subtract, op1=mybir.AluOpType.mult,
                                    )
                                else:
                                    # ACT: out = rstd*v + (-mean*rstd)
                                    nc.vector.tensor_scalar(
                                        out=st[:, 3:4], in0=st[:, 0:1],
                                        scalar1=st[:, 2:3], scalar2=-1.0,
                                        op0=mybir.AluOpType.mult, op1=mybir.AluOpType.mult,
                                    )
                                    nc.scalar.activation(
                                        out=vo[:, :], in_=v[:, :],
                                        func=mybir.ActivationFunctionType.Identity,
                                        bias=st[:, 3:4], scale=st[:, 2:3],
                                    )
                            else:
                                # out = ((v - mean)*gamma)*rstd + beta  (2 fused DVE ops)
                                nc.vector.scalar_tensor_tensor(
                                    out=vo[:, :], in0=v[:, :], scalar=st[:, 0:1],
                                    in1=gam[:, :],
                                    op0=mybir.AluOpType.subtract, op1=mybir.AluOpType.mult,
                                )
                                nc.vector.scalar_tensor_tensor(
                                    out=vo[:, :], in0=vo[:, :], scalar=st[:, 2:3],
                                    in1=bet[:, :],
                                    op0=mybir.AluOpType.mult, op1=mybir.AluOpType.add,
                                )
                            nc.sync.dma_start(out=out_d[128 * qp:128 * (qp + 1), :], in_=vo[:, :])
    nc.compile()
    return nc


_NC_CACHE = {}


def make_in_maps(inputs):
    x = np.asarray(inputs["inputs"], np.float32)
    wo = np.asarray(inputs["wo"], np.float32)
    f8 = lambda a: np.clip(np.ascontiguousarray(a), -240, 240).astype(ml_dtypes.float8_e4m3)
    bo_eff = np.asarray(inputs["bo"], np.float32) + np.asarray(inputs["bv"], np.float32) @ wo
    shared = {
        "wq": f8(inputs["wq"]), "wk": f8(inputs["wk"]),
        "wv": f8(inputs["wv"]), "wo": f8(wo),
        "bqT": np.ascontiguousarray(np.asarray(inputs["bq"], np.float32).reshape(NDC, 128).T),
        "bkT": np.ascontiguousarray(np.asarray(inputs["bk"], np.float32).reshape(NDC, 128).T),
        "gamma": np.asarray(inputs["gamma"], np.float32),
        "beta": np.asarray(inputs["beta"], np.float32),
    }
    in_maps = []
    for c in range(8):
        b, qh = c // 2, c % 2
        xslab = x[b, Q * qh:Q * (qh + 1), :]
        in_maps.append({
            **shared,
            "xt8": f8(xslab.T),
            "xq": (np.ascontiguousarray(xslab) + bo_eff[None, :]).astype(ml_dtypes.bfloat16),
        })
    return in_maps


def kernel(**inputs) -> np.ndarray:
    from concourse.bass_utils import run_bass_kernel_spmd

    trivial = bool(
        np.all(np.asarray(inputs["gamma"], np.float32) == 1.0)
        and np.all(np.asarray(inputs["beta"], np.float32) == 0.0)
    )
    if trivial not in _NC_CACHE:
        _NC_CACHE[trivial] = build_nc(trivial_gb=trivial)
    res = run_bass_kernel_spmd(_NC_CACHE[trivial], make_in_maps(inputs), core_ids=list(range(8)))
    out = np.empty((B, S, D), np.float32)
    for c in range(8):
        b, qh = c // 2, c % 2
        out[b, Q * qh:Q * (qh + 1), :] = res.results[c]["out"]
    return out


# revision 34
# speedup vs baseline: 1.0217x; 1.0217x over previous
"""Trainium2 Bass kernel for full-embed-dim self-attention + residual LayerNorm.

Problem: B=4, S=2048, D=1024 fp32.
  q/k/v = x@w{q,k,v}+b; scores = q@k^T/sqrt(D); attn = softmax(scores)@v;
  out = LN(x + attn@wo + bo) * gamma + beta.

Sharding: 8 cores = 4 batches x 2 query-halves (1024 queries each). Each
core computes K/V projections only for its own 1024 keys, then a pair-wise
AllGather ([0,1],[2,3],...) exchanges the halves so each core attends over
the batch's full 2048-key sequence.

All four matmul stages run in fp8(e4m3) DoubleRow mode: operands are packed
as [128, 2, N] pair tiles (two 128-row contraction chunks side by side), so
each matmul contracts 256 elements/instruction at 2 fp8 MACs/cell/cycle
(~1.9x measured over bf16; LDWEIGHTS fully pipelines behind the previous
matmul at 512-wide moving operands). PSUM accumulation stays fp32. e4m3
quantization noise (~3.6%/element) decorrelates over the 1024-2048-long
contractions; measured end-to-end max rel err is ~8e-3 vs the 2e-2 gate.
NOTE: DoubleRow weight APs require the pair-dim byte stride to be a
multiple of 16 - a 1-byte-stride ones vector hangs the PE (hence the
[128, 2, 16] ones tile).

Phase order (hides the two collectives behind compute; the CC engine has
a fixed ~50us init floor, so partner data can never arrive early -- the
schedule keeps the PE busy with own-half work until it does):
  KT proj -> AG-KT -> V proj -> AG-V -> QT proj -> scores over own keys ->
  attnV over own keys (m 0..3) into bf16 partials while the gathers land ->
  scores over partner keys -> denominators -> attnV over partner keys
  (drain adds the bf16 partial) interleaved with out-proj by query half so
  the LayerNorm epilogue of half 0 overlaps half 1's matmuls.

Per-core dataflow (d-on-partitions "transposed" layout throughout):
  QT[d,q]   = wq^T @ xq^T (+bq)      (host ships x^T fp8 for the core's half)
  KT[d,k_own] = wk^T @ xq^T (+bk)    --> AllGather --> KT full
  V[k_own,d]  = x @ wv               --> AllGather --> V full
  ST[k,q]   = KT^T @ QT              (contracted over d in PSUM)
  PT[k,q]   = exp(ST/32 - ln16)      (the /16 keeps PT in [0,11] and AT in
              ~[-45,45], inside e4m3's +-240 range; it cancels in the
              normalization since the denominator uses the same PT)
  AT[d,q]   = V^T @ PT ; denom[q] = ones^T @ PT  (DoubleRow ones vector)
  O[q,e]    = AT^T @ wo, then O/denom[q] + xq_aug (host adds bo+bv@wo,
              ships x+bo_eff as bf16), LayerNorm with the elementwise pass
              alternating between DVE and ACT per query block. gamma==1 /
              beta==0 (always true for this problem) takes a fused
              single-pass path; a general two-pass path is kept otherwise.
"""

import numpy as np
import ml_dtypes

import concourse.bass as bass
import concourse.mybir as mybir
import concourse.tile as tile
from concourse import bacc

F32 = mybir.dt.float32
F8 = mybir.dt.float8e4
BF16 = mybir.dt.bfloat16
DR = mybir.MatmulPerfMode.DoubleRow

B, S, D = 4, 2048, 1024
Q = 1024          # queries (and own keys) per core
SCALE = 1.0 / 32.0
NLOG16 = -2.772588722239781   # -ln(16): PT = exp(s)/16
EPS = 1e-6
NP = 4            # d pair-chunks (256 each)
NDC = 8           # 128-wide chunks per 1024
RG = [[0, 1], [2, 3], [4, 5], [6, 7]]


def _bcast_ap(ap_1d, parts=128):
    """[N] dram AP -> [parts, N] AP with 0-stride partition dim."""
    return bass.AP(
        tensor=ap_1d.tensor, offset=ap_1d.offset, ap=[[0, parts]] + list(ap_1d.ap)
    )


def build_nc(trivial_gb: bool = True):
    nc = bacc.Bacc("TRN2", target_bir_lowering=False, debug=False, num_devices=8)

    xt8_d = nc.dram_tensor("xt8", [D, Q], F8, kind="ExternalInput")
    xq_d = nc.dram_tensor("xq", [Q, D], BF16, kind="ExternalInput")
    wq_d = nc.dram_tensor("wq", [D, D], F8, kind="ExternalInput")
    wk_d = nc.dram_tensor("wk", [D, D], F8, kind="ExternalInput")
    wv_d = nc.dram_tensor("wv", [D, D], F8, kind="ExternalInput")
    wo_d = nc.dram_tensor("wo", [D, D], F8, kind="ExternalInput")
    bqT_d = nc.dram_tensor("bqT", [128, NDC], F32, kind="ExternalInput")
    bkT_d = nc.dram_tensor("bkT", [128, NDC], F32, kind="ExternalInput")
    gamma_d = nc.dram_tensor("gamma", [D], F32, kind="ExternalInput")
    beta_d = nc.dram_tensor("beta", [D], F32, kind="ExternalInput")
    out_d = nc.dram_tensor("out", [Q, D], BF16, kind="ExternalOutput")

    with tile.TileContext(nc) as tc:
        with (
            tc.tile_pool(name="small", bufs=1) as p_small,
            tc.tile_pool(name="dram", bufs=1, space="DRAM") as p_dram,
            tc.tile_pool(name="qtp", bufs=NP) as p_qt,
            tc.tile_pool(name="ktp", bufs=NP) as p_kt,
            tc.tile_pool(name="ktlp", bufs=NP) as p_ktl,
            tc.tile_pool(name="vp", bufs=NDC) as p_v,
            tc.tile_pool(name="ptp", bufs=NDC) as p_pt,
            tc.tile_pool(name="atp", bufs=NP) as p_at,
        ):
            kvin_kt = p_dram.tile([D, Q], F8, name="kvin_kt")
            kvout_kt = p_dram.tile([2 * D, Q], F8, name="kvout_kt")
            kvin_v = p_dram.tile([Q, D], F8, name="kvin_v")
            kvout_v = p_dram.tile([S, D], F8, name="kvout_v")

            # ---- constants / small tiles ----
            bqT = p_small.tile([128, NDC], F32)
            nc.gpsimd.dma_start(out=bqT[:, :], in_=bqT_d[:, :])
            bkT = p_small.tile([128, NDC], F32)
            nc.gpsimd.dma_start(out=bkT[:, :], in_=bkT_d[:, :])
            # [128, 2, 16] so the DoubleRow pair dim strides 16 B (HW requires
            # weight-AP step % 16 == 0); only column 0 is used as the ones vector.
            ones = p_small.tile([128, 2, 16], F8)
            nc.vector.memset(ones[:, :, :], 1.0)
            nl16 = p_small.tile([128, 1], F32)
            nc.vector.memset(nl16[:, :], NLOG16)
            eps_t = p_small.tile([128, 1], F32)
            nc.vector.memset(eps_t[:, :], EPS)
            recip = p_small.tile([128, 8], F32)

            # PE clock warmup: the array runs at ~half clock for the first
            # ~3us of activity; burn the ramp on junk matmuls while the first
            # weight/x DMAs are in flight.
            junk = p_small.tile([128, 2, 512], F8)
            nc.vector.memset(junk[:, :, :], 0.0)

            pid = nc.sync.partition_id()
            partner_off = (1 - (pid % 2)) * Q   # partner's row base in gathered buffers
            poff512 = (1 - (pid % 2)) * 512     # same, within a half-gather block

            qtp = [p_qt.tile([128, 2, Q], F8, tag="qt", name=f"qtp{j}") for j in range(NP)]
            ktp = [p_kt.tile([128, 2, Q], F8, tag="kt", name=f"ktp{j}") for j in range(NP)]
            ptp = [p_pt.tile([128, 2, Q], F8, tag="pt", name=f"ptp{m}") for m in range(NDC)]

            with (
                tc.tile_pool(name="wp", bufs=3 * NP) as p_w,
                tc.tile_pool(name="xp", bufs=NP) as p_x,
            ):
                wkp = [p_w.tile([128, 2, D], F8, tag="w", name=f"wkp{j}") for j in range(NP)]
                wqp = [p_w.tile([128, 2, D], F8, tag="w", name=f"wqp{j}") for j in range(NP)]
                wvp = [p_w.tile([128, 2, D], F8, tag="w", name=f"wvp{j}") for j in range(NP)]
                xp = [p_x.tile([128, 2, Q], F8, tag="x", name=f"xp{j}") for j in range(NP)]
                for j in range(NP):
                    for i in range(2):
                        r = 256 * j + 128 * i
                        xe = nc.sync if i == 0 else nc.scalar
                        we = nc.gpsimd if i == 0 else nc.scalar
                        xe.dma_start(out=xp[j][:, i, :], in_=xt8_d[r:r + 128, :])
                        we.dma_start(out=wkp[j][:, i, :], in_=wk_d[r:r + 128, :])
                for j in range(NP):
                    for i in range(2):
                        r = 256 * j + 128 * i
                        nc.sync.dma_start(out=wqp[j][:, i, :], in_=wq_d[r:r + 128, :])
                        nc.gpsimd.dma_start(out=wvp[j][:, i, :], in_=wv_d[r:r + 128, :])

                with tc.tile_pool(name="psp", bufs=6, space="PSUM") as p_ps_proj:
                    ps_warm = p_ps_proj.tile([128, 512], F32, tag="ps", name="ps_warm")
                    for w in range(12):
                        nc.tensor.matmul(
                            ps_warm[:, :], junk[:, :, 0:128], junk[:, :, :],
                            start=True, stop=True, perf_mode=DR,
                        )

                    # ---- KT_own[d, k_own] (+bk) -> ktp pair tiles + DRAM bounce ----
                    for do in range(NDC):
                        pss = [p_ps_proj.tile([128, 512], F32, tag="ps", name=f"pskt{do}_{h}") for h in range(2)]
                        for j in range(NP):
                            for kh in range(2):
                                nc.tensor.matmul(
                                    pss[kh][:, :],
                                    wkp[j][:, :, 128 * do:128 * (do + 1)],
                                    xp[j][:, :, 512 * kh:512 * (kh + 1)],
                                    start=(j == 0), stop=(j == NP - 1), perf_mode=DR,
                                )
                        for kh in range(2):
                            nc.vector.tensor_scalar(
                                out=ktp[do // 2][:, do % 2, 512 * kh:512 * (kh + 1)],
                                in0=pss[kh][:, :],
                                scalar1=bkT[:, do:do + 1], scalar2=None,
                                op0=mybir.AluOpType.add,
                            )
                    # Exchange KT in two d-halves. Stage ALL kvin stores first:
                    # putting the half-B stores after the ktlp-A loads would
                    # head-of-line-block them on the sync queue behind the AG-A
                    # completion wait, serializing half B behind half A.
                    ktlp = [p_ktl.tile([128, 2, Q], F8, tag="ktl", name=f"ktlp{j}") for j in range(NP)]
                    for do in range(NDC):
                        nc.sync.dma_start(
                            out=kvin_kt[128 * do:128 * (do + 1), :],
                            in_=ktp[do // 2][:, do % 2, :],
                        )
                    nc.gpsimd.collective_compute(
                        "AllGather", mybir.AluOpType.bypass, replica_groups=RG,
                        ins=[kvin_kt[:, :].opt()], outs=[kvout_kt[:, :].opt()],
                    )
                    # partner KT via runtime-parity offset
                    for j in range(NP):
                        for i in range(2):
                            nc.sync.dma_start(
                                out=ktlp[j][:, i, :],
                                in_=kvout_kt[bass.ds(partner_off + 256 * j + 128 * i, 128), :],
                            )

                    # ---- V_own[k_own, e] -> vp pair tiles (m 0..3) + DRAM bounce ----
                    vp = [p_v.tile([128, 2, D], F8, tag="v", name=f"vp{m}") for m in range(NDC)]
                    for kl in range(NDC):
                        pss = [p_ps_proj.tile([128, 512], F32, tag="ps", name=f"psv{kl}_{h}") for h in range(2)]
                        for j in range(NP):
                            for dh in range(2):
                                nc.tensor.matmul(
                                    pss[dh][:, :],
                                    xp[j][:, :, 128 * kl:128 * (kl + 1)],
                                    wvp[j][:, :, 512 * dh:512 * (dh + 1)],
                                    start=(j == 0), stop=(j == NP - 1), perf_mode=DR,
                                )
                        for dh in range(2):
                            nc.scalar.activation(
                                out=vp[kl // 2][:, kl % 2, 512 * dh:512 * (dh + 1)],
                                in_=pss[dh][:, :],
                                func=mybir.ActivationFunctionType.Copy,
                                bias=0.0, scale=1.0,
                            )
                    for kl in range(NDC):
                        nc.gpsimd.dma_start(
                            out=kvin_v[128 * kl:128 * (kl + 1), :],
                            in_=vp[kl // 2][:, kl % 2, :],
                        )
                    nc.gpsimd.collective_compute(
                        "AllGather", mybir.AluOpType.bypass, replica_groups=RG,
                        ins=[kvin_v[:, :].opt()], outs=[kvout_v[:, :].opt()],
                    )
                    # partner V (local key chunks 8..15)
                    for m in range(4, NDC):
                        for i in range(2):
                            nc.sync.dma_start(
                                out=vp[m][:, i, :],
                                in_=kvout_v[bass.ds(partner_off + 256 * (m - 4) + 128 * i, 128), :],
                            )

                    # ---- QT[d,q] (+bq) ----
                    for do in range(NDC):
                        pss = [p_ps_proj.tile([128, 512], F32, tag="ps", name=f"psqt{do}_{h}") for h in range(2)]
                        for j in range(NP):
                            for qh in range(2):
                                nc.tensor.matmul(
                                    pss[qh][:, :],
                                    wqp[j][:, :, 128 * do:128 * (do + 1)],
                                    xp[j][:, :, 512 * qh:512 * (qh + 1)],
                                    start=(j == 0), stop=(j == NP - 1), perf_mode=DR,
                                )
                        for qh in range(2):
                            nc.vector.tensor_scalar(
                                out=qtp[do // 2][:, do % 2, 512 * qh:512 * (qh + 1)],
                                in0=pss[qh][:, :],
                                scalar1=bqT[:, do:do + 1], scalar2=None,
                                op0=mybir.AluOpType.add,
                            )


            with (
                tc.tile_pool(name="ps", bufs=6, space="PSUM") as p_ps,
                tc.tile_pool(name="ps1", bufs=2, space="PSUM") as p_ps1,
            ):
                # bf16 partials for the own-key half of attnV, accumulated while
                # the KT/V gathers are still in flight
                ato = [p_at.tile([128, 2, Q], BF16, tag="ato", name=f"ato{j}") for j in range(NP)]

                # ---- ST -> exp -> PT; local key order: kc 0..7 own, 8..15 partner ----
                for kc in list(range(8)) + ["attnv_own"] + list(range(8, 16)):
                    if kc == "attnv_own":
                        # attnV over OWN keys (m 0..3) -> bf16 partials; fills
                        # the PE while AG-KT/AG-V complete
                        for qh in range(2):
                            for dc in range(NDC):
                                ps = p_ps.tile([128, 512], F32, tag="ps", name=f"psao{qh}_{dc}")
                                for m in range(4):
                                    nc.tensor.matmul(
                                        ps[:, :],
                                        vp[m][:, :, 128 * dc:128 * (dc + 1)],
                                        ptp[m][:, :, 512 * qh:512 * (qh + 1)],
                                        start=(m == 0), stop=(m == 3), perf_mode=DR,
                                    )
                                nc.vector.tensor_copy(
                                    ato[dc // 2][:, dc % 2, 512 * qh:512 * (qh + 1)], ps[:, :]
                                )
                        continue
                    kt_j = ktp if kc < NDC else ktlp
                    kcl = kc % NDC
                    pss = [p_ps.tile([128, 512], F32, tag="ps", name=f"psst{kc}_{h}") for h in range(2)]
                    for j in range(NP):
                        for qh in range(2):
                            nc.tensor.matmul(
                                pss[qh][:, :],
                                kt_j[j][:, :, 128 * kcl:128 * (kcl + 1)],
                                qtp[j][:, :, 512 * qh:512 * (qh + 1)],
                                start=(j == 0), stop=(j == NP - 1), perf_mode=DR,
                            )
                    for qh in range(2):
                        nc.scalar.activation(
                            out=ptp[kc // 2][:, kc % 2, 512 * qh:512 * (qh + 1)],
                            in_=pss[qh][:, :],
                            func=mybir.ActivationFunctionType.Exp,
                            bias=nl16[:, :], scale=SCALE,
                        )

                # ---- denominators: denom[q] = ones^T @ PT ----
                for qp in range(8):
                    ps1 = p_ps1.tile([128, 1], F32, tag="ps1", name=f"ps1_{qp}")
                    for m in range(NDC):
                        nc.tensor.matmul(
                            ps1[:, :],
                            ptp[m][:, :, 128 * qp:128 * (qp + 1)],
                            ones[:, :, 0:1],
                            start=(m == 0), stop=(m == NDC - 1), perf_mode=DR,
                        )
                    nc.vector.reciprocal(recip[:, qp:qp + 1], ps1[:, :])

                # ---- AT[d,q] = V^T @ PT by query-half, with the output
                # projection + LayerNorm for that half interleaved so the
                # elementwise epilogue overlaps the other half's matmuls ----
                atp = [p_at.tile([128, 2, Q], F8, tag="at", name=f"atp{j}") for j in range(NP)]
                with (
                    tc.tile_pool(name="wop", bufs=NP) as p_wo,
                    tc.tile_pool(name="xqp", bufs=4) as p_xq,
                    tc.tile_pool(name="vout", bufs=6) as p_vo,
                    tc.tile_pool(name="lnst", bufs=8) as p_ln,
                ):
                    wop = [p_wo.tile([128, 2, D], F8, tag="wo", name=f"wop{j}") for j in range(NP)]
                    for j in range(NP):
                        for i in range(2):
                            r = 256 * j + 128 * i
                            nc.scalar.dma_start(out=wop[j][:, i, :], in_=wo_d[r:r + 128, :])
                    gam = p_small.tile([128, D], F32)
                    bet = p_small.tile([128, D], F32)
                    if not trivial_gb:
                        nc.gpsimd.dma_start(out=gam[:, :], in_=_bcast_ap(gamma_d[:]))
                        nc.gpsimd.dma_start(out=bet[:, :], in_=_bcast_ap(beta_d[:]))

                    for qh in range(2):
                        for dc in range(NDC):
                            ps = p_ps.tile([128, 512], F32, tag="ps", name=f"psat{qh}_{dc}")
                            for m in range(4, NDC):
                                nc.tensor.matmul(
                                    ps[:, :],
                                    vp[m][:, :, 128 * dc:128 * (dc + 1)],
                                    ptp[m][:, :, 512 * qh:512 * (qh + 1)],
                                    start=(m == 4), stop=(m == NDC - 1), perf_mode=DR,
                                )
                            # AT = partner partial (psum) + own partial (bf16)
                            nc.vector.tensor_add(
                                atp[dc // 2][:, dc % 2, 512 * qh:512 * (qh + 1)],
                                ps[:, :],
                                ato[dc // 2][:, dc % 2, 512 * qh:512 * (qh + 1)],
                            )

                        for qp in range(4 * qh, 4 * qh + 4):
                            v = p_vo.tile([128, D], BF16, tag="v", name=f"v{qp}")
                            sqs = p_vo.tile([128, D], BF16, tag="sqs", name=f"sqs{qp}")
                            xqt_ = p_xq.tile([128, D], BF16, tag="xq", name=f"xqt{qp}")
                            nc.scalar.dma_start(
                                out=xqt_[:, :], in_=xq_d[128 * qp:128 * (qp + 1), :]
                            )
                            st = p_ln.tile([128, 8], F32, tag="st", name=f"st{qp}")
                            for eh in range(2):
                                ps = p_ps.tile([128, 512], F32, tag="ps", name=f"pso{qp}_{eh}")
                                for j in range(NP):
                                    nc.tensor.matmul(
                                        ps[:, :],
                                        atp[j][:, :, 128 * qp:128 * (qp + 1)],
                                        wop[j][:, :, 512 * eh:512 * (eh + 1)],
                                        start=(j == 0), stop=(j == NP - 1), perf_mode=DR,
                                    )
                                # v_half = O/denom + xq_aug; accum = row-sum
                                nc.vector.scalar_tensor_tensor(
                                    out=v[:, 512 * eh:512 * (eh + 1)], in0=ps[:, :],
                                    scalar=recip[:, qp:qp + 1],
                                    in1=xqt_[:, 512 * eh:512 * (eh + 1)],
                                    op0=mybir.AluOpType.mult, op1=mybir.AluOpType.add,
                                    accum_out=st[:, eh:eh + 1],
                                )
                            # E[v^2] via ACT Square + free accum; then mean/var/rstd
                            nc.scalar.activation(
                                out=sqs[:, :], in_=v[:, :],
                                func=mybir.ActivationFunctionType.Square,
                                accum_out=st[:, 2:3],
                            )
                            # mean = (s0+s1)/D ; var = sq/D - mean^2
                            nc.vector.tensor_scalar(
                                out=st[:, 0:1], in0=st[:, 0:1],
                                scalar1=st[:, 1:2], scalar2=1.0 / D,
                                op0=mybir.AluOpType.add, op1=mybir.AluOpType.mult,
                            )
                            nc.vector.tensor_mul(st[:, 1:2], st[:, 0:1], st[:, 0:1])
                            nc.vector.tensor_scalar(
                                out=st[:, 2:3], in0=st[:, 2:3],
                                scalar1=1.0 / D, scalar2=st[:, 1:2],
                                op0=mybir.AluOpType.mult, op1=mybir.AluOpType.subtract,
                            )
                            nc.scalar.activation(
                                out=st[:, 2:3], in_=st[:, 2:3],
                                func=mybir.ActivationFunctionType.Sqrt,
                                bias=eps_t[:, :],
                            )
                            nc.vector.reciprocal(st[:, 2:3], st[:, 2:3])       # rstd
                            vo = p_vo.tile([128, D], BF16, tag="vo", name=f"vo{qp}")
                            if trivial_gb:
                                # gamma==1, beta==0: out = (v - mean)*rstd in one
                                # pass, alternating DVE / ACT per block.
                                if qp % 2 == 0:
                                    nc.vector.tensor_scalar(
                                        out=vo[:, :], in0=v[:, :],
                                        scalar1=st[:, 0:1], scalar2=st[:, 2:3],
                                        op0=mybir.AluOpType.subtract, op1=mybir.AluOpType.mult,
                                    )
                                else:
                                    # ACT: out = rstd*v + (-mean*rstd)
                                    nc.vector.tensor_scalar(
                                        out=st[:, 3:4], in0=st[:, 0:1],
                                        scalar1=st[:, 2:3], scalar2=-1.0,
                                        op0=mybir.AluOpType.mult, op1=mybir.AluOpType.mult,
                                    )
                                    nc.scalar.activation(
                                        out=vo[:, :], in_=v[:, :],
                                        func=mybir.ActivationFunctionType.Identity,
                                        bias=st[:, 3:4], scale=st[:, 2:3],
                                    )
                            else:
                                # out = ((v - mean)*gamma)*rstd + beta  (2 fused DVE ops)
                                nc.vector.scalar_tensor_tensor(
                                    out=vo[:, :], in0=v[:, :], scalar=st[:, 0:1],
                                    in1=gam[:, :],
                                    op0=mybir.AluOpType.subtract, op1=mybir.AluOpType.mult,
                                )
                                nc.vector.scalar_tensor_tensor(
                                    out=vo[:, :], in0=vo[:, :], scalar=st[:, 2:3],
                                    in1=bet[:, :],
                                    op0=mybir.AluOpType.mult, op1=mybir.AluOpType.add,
                                )
                            nc.sync.dma_start(out=out_d[128 * qp:128 * (qp + 1), :], in_=vo[:, :])
    nc.compile()
    return nc


_NC_CACHE = {}


def make_in_maps(inputs):
    x = np.asarray(inputs["inputs"], np.float32)
    wo = np.asarray(inputs["wo"], np.float32)
    f8 = lambda a: np.clip(np.ascontiguousarray(a), -240, 240).astype(ml_dtypes.float8_e4m3)
    bo_eff = np.asarray(inputs["bo"], np.float32) + np.asarray(inputs["bv"], np.float32) @ wo
    shared = {
        "wq": f8(inputs["wq"]), "wk": f8(inputs["wk"]),
        "wv": f8(inputs["wv"]), "wo": f8(wo),
        "bqT": np.ascontiguousarray(np.asarray(inputs["bq"], np.float32).reshape(NDC, 128).T),
        "bkT": np.ascontiguousarray(np.asarray(inputs["bk"], np.float32).reshape(NDC, 128).T),
        "gamma": np.asarray(inputs["gamma"], np.float32),
        "beta": np.asarray(inputs["beta"], np.float32),
    }
    in_maps = []
    for c in range(8):
        b, qh = c // 2, c % 2
        xslab = x[b, Q * qh:Q * (qh + 1), :]
        in_maps.append({
            **shared,
            "xt8": f8(xslab.T),
            "xq": (np.ascontiguousarray(xslab) + bo_eff[None, :]).astype(ml_dtypes.bfloat16),
        })
    return in_maps


def kernel(**inputs) -> np.ndarray:
    from concourse.bass_utils import run_bass_kernel_spmd

    trivial = bool(
        np.all(np.asarray(inputs["gamma"], np.float32) == 1.0)
        and np.all(np.asarray(inputs["beta"], np.float32) == 0.0)
    )
    if trivial not in _NC_CACHE:
        _NC_CACHE[trivial] = build_nc(trivial_gb=trivial)
    res = run_bass_kernel_spmd(_NC_CACHE[trivial], make_in_maps(inputs), core_ids=list(range(8)))
    out = np.empty((B, S, D), np.float32)
    for c in range(8):
        b, qh = c // 2, c % 2
        out[b, Q * qh:Q * (qh + 1), :] = res.results[c]["out"].astype(np.float32)
    return out


# revision 35
# speedup vs baseline: 1.0577x; 1.0353x over previous
"""Trainium2 Bass kernel for full-embed-dim self-attention + residual LayerNorm.

Problem: B=4, S=2048, D=1024 fp32.
  q/k/v = x@w{q,k,v}+b; scores = q@k^T/sqrt(D); attn = softmax(scores)@v;
  out = LN(x + attn@wo + bo) * gamma + beta.

Sharding: 8 cores = 4 batches x 2 query-halves (1024 queries each). Each
core computes K/V projections only for its own 1024 keys, then a pair-wise
AllGather ([0,1],[2,3],...) exchanges the halves so each core attends over
the batch's full 2048-key sequence.

All four matmul stages run in fp8(e4m3) DoubleRow mode: operands are packed
as [128, 2, N] pair tiles (two 128-row contraction chunks side by side), so
each matmul contracts 256 elements/instruction at 2 fp8 MACs/cell/cycle
(~1.9x measured over bf16; LDWEIGHTS fully pipelines behind the previous
matmul at 512-wide moving operands). PSUM accumulation stays fp32. e4m3
quantization noise (~3.6%/element) decorrelates over the 1024-2048-long
contractions; measured end-to-end max rel err is ~8e-3 vs the 2e-2 gate.
NOTE: DoubleRow weight APs require the pair-dim byte stride to be a
multiple of 16 - a 1-byte-stride ones vector hangs the PE (hence the
[128, 2, 16] ones tile).

Phase order (hides the two collectives behind compute; the CC engine has
a fixed ~50us init floor, so partner data can never arrive early -- the
schedule keeps the PE busy with own-half work until it does):
  KT proj -> AG-KT -> V proj -> AG-V -> QT proj -> scores over own keys ->
  attnV over own keys (m 0..3) into bf16 partials while the gathers land ->
  scores over partner keys -> denominators -> attnV over partner keys
  (drain adds the bf16 partial) interleaved with out-proj by query half so
  the LayerNorm epilogue of half 0 overlaps half 1's matmuls.

Per-core dataflow (d-on-partitions "transposed" layout throughout):
  QT[d,q]   = wq^T @ xq^T (+bq)      (host ships x^T fp8 for the core's half)
  KT[d,k_own] = wk^T @ xq^T (+bk)    --> AllGather --> KT full
  V[k_own,d]  = x @ wv               --> AllGather --> V full
  ST[k,q]   = KT^T @ QT              (contracted over d in PSUM)
  PT[k,q]   = exp(ST/32 - ln16)      (the /16 keeps PT in [0,11] and AT in
              ~[-45,45], inside e4m3's +-240 range; it cancels in the
              normalization since the denominator uses the same PT)
  AT[d,q]   = V^T @ PT ; denom[q] = ones^T @ PT  (DoubleRow ones vector)
  O[q,e]    = AT^T @ wo, then O/denom[q] + xq_aug (host adds bo+bv@wo,
              ships x+bo_eff as bf16), LayerNorm with the elementwise pass
              alternating between DVE and ACT per query block. gamma==1 /
              beta==0 (always true for this problem) takes a fused
              single-pass path; a general two-pass path is kept otherwise.
"""

import numpy as np
import ml_dtypes

import concourse.bass as bass
import concourse.mybir as mybir
import concourse.tile as tile
from concourse import bacc

F32 = mybir.dt.float32
F8 = mybir.dt.float8e4
BF16 = mybir.dt.bfloat16
DR = mybir.MatmulPerfMode.DoubleRow

B, S, D = 4, 2048, 1024
Q = 1024          # queries (and own keys) per core
SCALE = 1.0 / 32.0
NLOG16 = -2.772588722239781   # -ln(16): PT = exp(s)/16
EPS = 1e-6
NP = 4            # d pair-chunks (256 each)
NDC = 8           # 128-wide chunks per 1024
RG = [[0, 1], [2, 3], [4, 5], [6, 7]]


def _bcast_ap(ap_1d, parts=128):
    """[N] dram AP -> [parts, N] AP with 0-stride partition dim."""
    return bass.AP(
        tensor=ap_1d.tensor, offset=ap_1d.offset, ap=[[0, parts]] + list(ap_1d.ap)
    )


def build_nc(trivial_gb: bool = True):
    nc = bacc.Bacc("TRN2", target_bir_lowering=False, debug=False, num_devices=8)

    xt8_d = nc.dram_tensor("xt8", [D, Q], F8, kind="ExternalInput")
    xq_d = nc.dram_tensor("xq", [Q, D], BF16, kind="ExternalInput")
    wq_d = nc.dram_tensor("wq", [D, D], F8, kind="ExternalInput")
    wk_d = nc.dram_tensor("wk", [D, D], F8, kind="ExternalInput")
    wv_d = nc.dram_tensor("wv", [D, D], F8, kind="ExternalInput")
    wo_d = nc.dram_tensor("wo", [D, D], F8, kind="ExternalInput")
    bqT_d = nc.dram_tensor("bqT", [128, NDC], F32, kind="ExternalInput")
    bkT_d = nc.dram_tensor("bkT", [128, NDC], F32, kind="ExternalInput")
    gamma_d = nc.dram_tensor("gamma", [D], F32, kind="ExternalInput")
    beta_d = nc.dram_tensor("beta", [D], F32, kind="ExternalInput")
    out_d = nc.dram_tensor("out", [Q, D], BF16, kind="ExternalOutput")

    with tile.TileContext(nc) as tc:
        with (
            tc.tile_pool(name="small", bufs=1) as p_small,
            tc.tile_pool(name="dram", bufs=1, space="DRAM") as p_dram,
            tc.tile_pool(name="qtp", bufs=NP) as p_qt,
            tc.tile_pool(name="ktp", bufs=NP) as p_kt,
            tc.tile_pool(name="ktlp", bufs=NP) as p_ktl,
            tc.tile_pool(name="vp", bufs=NDC) as p_v,
            tc.tile_pool(name="ptp", bufs=NDC) as p_pt,
            tc.tile_pool(name="atp", bufs=NP) as p_at,
        ):
            kvin_kt = p_dram.tile([D, Q], F8, name="kvin_kt")
            kvout_kt = p_dram.tile([2 * D, Q], F8, name="kvout_kt")
            kvin_v = p_dram.tile([Q, D], F8, name="kvin_v")
            kvout_v = p_dram.tile([S, D], F8, name="kvout_v")

            # ---- constants / small tiles ----
            bqT = p_small.tile([128, NDC], F32)
            nc.gpsimd.dma_start(out=bqT[:, :], in_=bqT_d[:, :])
            bkT = p_small.tile([128, NDC], F32)
            nc.gpsimd.dma_start(out=bkT[:, :], in_=bkT_d[:, :])
            # [128, 2, 16] so the DoubleRow pair dim strides 16 B (HW requires
            # weight-AP step % 16 == 0); only column 0 is used as the ones vector.
            ones = p_small.tile([128, 2, 16], F8)
            nc.vector.memset(ones[:, :, :], 1.0)
            nl16 = p_small.tile([128, 1], F32)
            nc.vector.memset(nl16[:, :], NLOG16)
            eps_t = p_small.tile([128, 1], F32)
            nc.vector.memset(eps_t[:, :], EPS)
            recip = p_small.tile([128, 8], F32)

            pid = nc.sync.partition_id()
            partner_off = (1 - (pid % 2)) * Q   # partner's row base in gathered buffers
            poff512 = (1 - (pid % 2)) * 512     # same, within a half-gather block

            qtp = [p_qt.tile([128, 2, Q], F8, tag="qt", name=f"qtp{j}") for j in range(NP)]
            ktp = [p_kt.tile([128, 2, Q], F8, tag="kt", name=f"ktp{j}") for j in range(NP)]
            ptp = [p_pt.tile([128, 2, Q], F8, tag="pt", name=f"ptp{m}") for m in range(NDC)]

            with (
                tc.tile_pool(name="wp", bufs=3 * NP) as p_w,
                tc.tile_pool(name="xp", bufs=NP) as p_x,
            ):
                wkp = [p_w.tile([128, 2, D], F8, tag="w", name=f"wkp{j}") for j in range(NP)]
                wqp = [p_w.tile([128, 2, D], F8, tag="w", name=f"wqp{j}") for j in range(NP)]
                wvp = [p_w.tile([128, 2, D], F8, tag="w", name=f"wvp{j}") for j in range(NP)]
                xp = [p_x.tile([128, 2, Q], F8, tag="x", name=f"xp{j}") for j in range(NP)]
                for j in range(NP):
                    for i in range(2):
                        r = 256 * j + 128 * i
                        xe = nc.sync if i == 0 else nc.scalar
                        we = nc.gpsimd if i == 0 else nc.scalar
                        xe.dma_start(out=xp[j][:, i, :], in_=xt8_d[r:r + 128, :])
                        we.dma_start(out=wkp[j][:, i, :], in_=wk_d[r:r + 128, :])
                for j in range(NP):
                    for i in range(2):
                        r = 256 * j + 128 * i
                        nc.sync.dma_start(out=wqp[j][:, i, :], in_=wq_d[r:r + 128, :])
                        nc.gpsimd.dma_start(out=wvp[j][:, i, :], in_=wv_d[r:r + 128, :])

                with tc.tile_pool(name="psp", bufs=6, space="PSUM") as p_ps_proj:
                    # ---- KT_own[d, k_own] (+bk) -> ktp pair tiles + DRAM bounce ----
                    for do in range(NDC):
                        pss = [p_ps_proj.tile([128, 512], F32, tag="ps", name=f"pskt{do}_{h}") for h in range(2)]
                        for j in range(NP):
                            for kh in range(2):
                                nc.tensor.matmul(
                                    pss[kh][:, :],
                                    wkp[j][:, :, 128 * do:128 * (do + 1)],
                                    xp[j][:, :, 512 * kh:512 * (kh + 1)],
                                    start=(j == 0), stop=(j == NP - 1), perf_mode=DR,
                                )
                        for kh in range(2):
                            nc.vector.tensor_scalar(
                                out=ktp[do // 2][:, do % 2, 512 * kh:512 * (kh + 1)],
                                in0=pss[kh][:, :],
                                scalar1=bkT[:, do:do + 1], scalar2=None,
                                op0=mybir.AluOpType.add,
                            )
                    # Exchange KT in two d-halves. Stage ALL kvin stores first:
                    # putting the half-B stores after the ktlp-A loads would
                    # head-of-line-block them on the sync queue behind the AG-A
                    # completion wait, serializing half B behind half A.
                    ktlp = [p_ktl.tile([128, 2, Q], F8, tag="ktl", name=f"ktlp{j}") for j in range(NP)]
                    for do in range(NDC):
                        nc.sync.dma_start(
                            out=kvin_kt[128 * do:128 * (do + 1), :],
                            in_=ktp[do // 2][:, do % 2, :],
                        )
                    nc.gpsimd.collective_compute(
                        "AllGather", mybir.AluOpType.bypass, replica_groups=RG,
                        ins=[kvin_kt[:, :].opt()], outs=[kvout_kt[:, :].opt()],
                    )
                    # partner KT via runtime-parity offset
                    for j in range(NP):
                        for i in range(2):
                            nc.sync.dma_start(
                                out=ktlp[j][:, i, :],
                                in_=kvout_kt[bass.ds(partner_off + 256 * j + 128 * i, 128), :],
                            )

                    # ---- V_own[k_own, e] -> vp pair tiles (m 0..3) + DRAM bounce ----
                    vp = [p_v.tile([128, 2, D], F8, tag="v", name=f"vp{m}") for m in range(NDC)]
                    for kl in range(NDC):
                        pss = [p_ps_proj.tile([128, 512], F32, tag="ps", name=f"psv{kl}_{h}") for h in range(2)]
                        for j in range(NP):
                            for dh in range(2):
                                nc.tensor.matmul(
                                    pss[dh][:, :],
                                    xp[j][:, :, 128 * kl:128 * (kl + 1)],
                                    wvp[j][:, :, 512 * dh:512 * (dh + 1)],
                                    start=(j == 0), stop=(j == NP - 1), perf_mode=DR,
                                )
                        for dh in range(2):
                            nc.scalar.activation(
                                out=vp[kl // 2][:, kl % 2, 512 * dh:512 * (dh + 1)],
                                in_=pss[dh][:, :],
                                func=mybir.ActivationFunctionType.Copy,
                                bias=0.0, scale=1.0,
                            )
                    for kl in range(NDC):
                        nc.gpsimd.dma_start(
                            out=kvin_v[128 * kl:128 * (kl + 1), :],
                            in_=vp[kl // 2][:, kl % 2, :],
                        )
                    nc.gpsimd.collective_compute(
                        "AllGather", mybir.AluOpType.bypass, replica_groups=RG,
                        ins=[kvin_v[:, :].opt()], outs=[kvout_v[:, :].opt()],
                    )
                    # partner V (local key chunks 8..15)
                    for m in range(4, NDC):
                        for i in range(2):
                            nc.sync.dma_start(
                                out=vp[m][:, i, :],
                                in_=kvout_v[bass.ds(partner_off + 256 * (m - 4) + 128 * i, 128), :],
                            )

                    # ---- QT[d,q] (+bq) ----
                    for do in range(NDC):
                        pss = [p_ps_proj.tile([128, 512], F32, tag="ps", name=f"psqt{do}_{h}") for h in range(2)]
                        for j in range(NP):
                            for qh in range(2):
                                nc.tensor.matmul(
                                    pss[qh][:, :],
                                    wqp[j][:, :, 128 * do:128 * (do + 1)],
                                    xp[j][:, :, 512 * qh:512 * (qh + 1)],
                                    start=(j == 0), stop=(j == NP - 1), perf_mode=DR,
                                )
                        for qh in range(2):
                            nc.vector.tensor_scalar(
                                out=qtp[do // 2][:, do % 2, 512 * qh:512 * (qh + 1)],
                                in0=pss[qh][:, :],
                                scalar1=bqT[:, do:do + 1], scalar2=None,
                                op0=mybir.AluOpType.add,
                            )


            with (
                tc.tile_pool(name="ps", bufs=6, space="PSUM") as p_ps,
                tc.tile_pool(name="ps1", bufs=2, space="PSUM") as p_ps1,
            ):
                # bf16 partials for the own-key half of attnV, accumulated while
                # the KT/V gathers are still in flight
                ato = [p_at.tile([128, 2, Q], BF16, tag="ato", name=f"ato{j}") for j in range(NP)]

                # ---- ST -> exp -> PT; local key order: kc 0..7 own, 8..15 partner ----
                for kc in list(range(8)) + ["attnv_own"] + list(range(8, 16)):
                    if kc == "attnv_own":
                        # attnV over OWN keys (m 0..3) -> bf16 partials; fills
                        # the PE while AG-KT/AG-V complete
                        for qh in range(2):
                            for dc in range(NDC):
                                ps = p_ps.tile([128, 512], F32, tag="ps", name=f"psao{qh}_{dc}")
                                for m in range(4):
                                    nc.tensor.matmul(
                                        ps[:, :],
                                        vp[m][:, :, 128 * dc:128 * (dc + 1)],
                                        ptp[m][:, :, 512 * qh:512 * (qh + 1)],
                                        start=(m == 0), stop=(m == 3), perf_mode=DR,
                                    )
                                nc.vector.tensor_copy(
                                    ato[dc // 2][:, dc % 2, 512 * qh:512 * (qh + 1)], ps[:, :]
                                )
                        continue
                    kt_j = ktp if kc < NDC else ktlp
                    kcl = kc % NDC
                    pss = [p_ps.tile([128, 512], F32, tag="ps", name=f"psst{kc}_{h}") for h in range(2)]
                    for j in range(NP):
                        for qh in range(2):
                            nc.tensor.matmul(
                                pss[qh][:, :],
                                kt_j[j][:, :, 128 * kcl:128 * (kcl + 1)],
                                qtp[j][:, :, 512 * qh:512 * (qh + 1)],
                                start=(j == 0), stop=(j == NP - 1), perf_mode=DR,
                            )
                    for qh in range(2):
                        nc.scalar.activation(
                            out=ptp[kc // 2][:, kc % 2, 512 * qh:512 * (qh + 1)],
                            in_=pss[qh][:, :],
                            func=mybir.ActivationFunctionType.Exp,
                            bias=nl16[:, :], scale=SCALE,
                        )

                # ---- denominators: denom[q] = ones^T @ PT ----
                for qp in range(8):
                    ps1 = p_ps1.tile([128, 1], F32, tag="ps1", name=f"ps1_{qp}")
                    for m in range(NDC):
                        nc.tensor.matmul(
                            ps1[:, :],
                            ptp[m][:, :, 128 * qp:128 * (qp + 1)],
                            ones[:, :, 0:1],
                            start=(m == 0), stop=(m == NDC - 1), perf_mode=DR,
                        )
                    nc.vector.reciprocal(recip[:, qp:qp + 1], ps1[:, :])

                # ---- AT[d,q] = V^T @ PT by query-half, with the output
                # projection + LayerNorm for that half interleaved so the
                # elementwise epilogue overlaps the other half's matmuls ----
                atp = [p_at.tile([128, 2, Q], F8, tag="at", name=f"atp{j}") for j in range(NP)]
                with (
                    tc.tile_pool(name="wop", bufs=NP) as p_wo,
                    tc.tile_pool(name="xqp", bufs=4) as p_xq,
                    tc.tile_pool(name="vout", bufs=6) as p_vo,
                    tc.tile_pool(name="lnst", bufs=8) as p_ln,
                ):
                    wop = [p_wo.tile([128, 2, D], F8, tag="wo", name=f"wop{j}") for j in range(NP)]
                    for j in range(NP):
                        for i in range(2):
                            r = 256 * j + 128 * i
                            nc.scalar.dma_start(out=wop[j][:, i, :], in_=wo_d[r:r + 128, :])
                    gam = p_small.tile([128, D], F32)
                    bet = p_small.tile([128, D], F32)
                    if not trivial_gb:
                        nc.gpsimd.dma_start(out=gam[:, :], in_=_bcast_ap(gamma_d[:]))
                        nc.gpsimd.dma_start(out=bet[:, :], in_=_bcast_ap(beta_d[:]))

                    for qh in range(2):
                        for dc in range(NDC):
                            ps = p_ps.tile([128, 512], F32, tag="ps", name=f"psat{qh}_{dc}")
                            for m in range(4, NDC):
                                nc.tensor.matmul(
                                    ps[:, :],
                                    vp[m][:, :, 128 * dc:128 * (dc + 1)],
                                    ptp[m][:, :, 512 * qh:512 * (qh + 1)],
                                    start=(m == 4), stop=(m == NDC - 1), perf_mode=DR,
                                )
                            # AT = partner partial (psum) + own partial (bf16)
                            nc.vector.tensor_add(
                                atp[dc // 2][:, dc % 2, 512 * qh:512 * (qh + 1)],
                                ps[:, :],
                                ato[dc // 2][:, dc % 2, 512 * qh:512 * (qh + 1)],
                            )

                        for qp in range(4 * qh, 4 * qh + 4):
                            v = p_vo.tile([128, D], BF16, tag="v", name=f"v{qp}")
                            sqs = p_vo.tile([128, D], BF16, tag="sqs", name=f"sqs{qp}")
                            xqt_ = p_xq.tile([128, D], BF16, tag="xq", name=f"xqt{qp}")
                            nc.scalar.dma_start(
                                out=xqt_[:, :], in_=xq_d[128 * qp:128 * (qp + 1), :]
                            )
                            st = p_ln.tile([128, 8], F32, tag="st", name=f"st{qp}")
                            for eh in range(2):
                                ps = p_ps.tile([128, 512], F32, tag="ps", name=f"pso{qp}_{eh}")
                                for j in range(NP):
                                    nc.tensor.matmul(
                                        ps[:, :],
                                        atp[j][:, :, 128 * qp:128 * (qp + 1)],
                                        wop[j][:, :, 512 * eh:512 * (eh + 1)],
                                        start=(j == 0), stop=(j == NP - 1), perf_mode=DR,
                                    )
                                # v_half = O/denom + xq_aug; accum = row-sum
                                nc.vector.scalar_tensor_tensor(
                                    out=v[:, 512 * eh:512 * (eh + 1)], in0=ps[:, :],
                                    scalar=recip[:, qp:qp + 1],
                                    in1=xqt_[:, 512 * eh:512 * (eh + 1)],
                                    op0=mybir.AluOpType.mult, op1=mybir.AluOpType.add,
                                    accum_out=st[:, eh:eh + 1],
                                )
                            # E[v^2] via ACT Square + free accum; then mean/var/rstd
                            nc.scalar.activation(
                                out=sqs[:, :], in_=v[:, :],
                                func=mybir.ActivationFunctionType.Square,
                                accum_out=st[:, 2:3],
                            )
                            # mean = (s0+s1)/D ; var = sq/D - mean^2
                            nc.vector.tensor_scalar(
                                out=st[:, 0:1], in0=st[:, 0:1],
                                scalar1=st[:, 1:2], scalar2=1.0 / D,
                                op0=mybir.AluOpType.add, op1=mybir.AluOpType.mult,
                            )
                            nc.vector.tensor_mul(st[:, 1:2], st[:, 0:1], st[:, 0:1])
                            nc.vector.tensor_scalar(
                                out=st[:, 2:3], in0=st[:, 2:3],
                                scalar1=1.0 / D, scalar2=st[:, 1:2],
                                op0=mybir.AluOpType.mult, op1=mybir.AluOpType.subtract,
                            )
                            nc.scalar.activation(
                                out=st[:, 2:3], in_=st[:, 2:3],
                                func=mybir.ActivationFunctionType.Sqrt,
                                bias=eps_t[:, :],
                            )
                            nc.vector.reciprocal(st[:, 2:3], st[:, 2:3])       # rstd
                            vo = p_vo.tile([128, D], BF16, tag="vo", name=f"vo{qp}")
                            if trivial_gb:
                                # gamma==1, beta==0: out = (v - mean)*rstd in one
                                # pass, alternating DVE / ACT per block.
                                if qp % 2 == 0:
                                    nc.vector.tensor_scalar(
                                        out=vo[:, :], in0=v[:, :],
                                        scalar1=st[:, 0:1], scalar2=st[:, 2:3],
                                        op0=mybir.AluOpType.subtract, op1=mybir.AluOpType.mult,
                                    )
                                else:
                                    # ACT: out = rstd*v + (-mean*rstd)
                                    nc.vector.tensor_scalar(
                                        out=st[:, 3:4], in0=st[:, 0:1],
                                        scalar1=st[:, 2:3], scalar2=-1.0,
                                        op0=mybir.AluOpType.mult, op1=mybir.AluOpType.mult,
                                    )
                                    nc.scalar.activation(
                                        out=vo[:, :], in_=v[:, :],
                                        func=mybir.ActivationFunctionType.Identity,
                                        bias=st[:, 3:4], scale=st[:, 2:3],
                                    )
                            else:
                                # out = ((v - mean)*gamma)*rstd + beta  (2 fused DVE ops)
                                nc.vector.scalar_tensor_tensor(
                                    out=vo[:, :], in0=v[:, :], scalar=st[:, 0:1],
                                    in1=gam[:, :],
                                    op0=mybir.AluOpType.subtract, op1=mybir.AluOpType.mult,
                                )
                                nc.vector.scalar_tensor_tensor(
                                    out=vo[:, :], in0=vo[:, :], scalar=st[:, 2:3],
                                    in1=bet[:, :],
                                    op0=mybir.AluOpType.mult, op1=mybir.AluOpType.add,
                                )
                            nc.sync.dma_start(out=out_d[128 * qp:128 * (qp + 1), :], in_=vo[:, :])
    nc.compile()
    return nc


_NC_CACHE = {}


def make_in_maps(inputs):
    x = np.asarray(inputs["inputs"], np.float32)
    wo = np.asarray(inputs["wo"], np.float32)
    f8 = lambda a: np.clip(np.ascontiguousarray(a), -240, 240).astype(ml_dtypes.float8_e4m3)
    bo_eff = np.asarray(inputs["bo"], np.float32) + np.asarray(inputs["bv"], np.float32) @ wo
    shared = {
        "wq": f8(inputs["wq"]), "wk": f8(inputs["wk"]),
        "wv": f8(inputs["wv"]), "wo": f8(wo),
        "bqT": np.ascontiguousarray(np.asarray(inputs["bq"], np.float32).reshape(NDC, 128).T),
        "bkT": np.ascontiguousarray(np.asarray(inputs["bk"], np.float32).reshape(NDC, 128).T),
        "gamma": np.asarray(inputs["gamma"], np.float32),
        "beta": np.asarray(inputs["beta"], np.float32),
    }
    in_maps = []
    for c in range(8):
        b, qh = c // 2, c % 2
        xslab = x[b, Q * qh:Q * (qh + 1), :]
        in_maps.append({
            **shared,
            "xt8": f8(xslab.T),
            "xq": (np.ascontiguousarray(xslab) + bo_eff[None, :]).astype(ml_dtypes.bfloat16),
        })
    return in_maps


def kernel(**inputs) -> np.ndarray:
    from concourse.bass_utils import run_bass_kernel_spmd

    trivial = bool(
        np.all(np.asarray(inputs["gamma"], np.float32) == 1.0)
        and np.all(np.asarray(inputs["beta"], np.float32) == 0.0)
    )
    if trivial not in _NC_CACHE:
        _NC_CACHE[trivial] = build_nc(trivial_gb=trivial)
    res = run_bass_kernel_spmd(_NC_CACHE[trivial], make_in_maps(inputs), core_ids=list(range(8)))
    out = np.empty((B, S, D), np.float32)
    for c in range(8):
        b, qh = c // 2, c % 2
        out[b, Q * qh:Q * (qh + 1), :] = res.results[c]["out"].astype(np.float32)
    return out


# revision 36
# speedup vs baseline: 1.0643x; 1.0061x over previous
"""Trainium2 Bass kernel for full-embed-dim self-attention + residual LayerNorm.

Problem: B=4, S=2048, D=1024 fp32.
  q/k/v = x@w{q,k,v}+b; scores = q@k^T/sqrt(D); attn = softmax(scores)@v;
  out = LN(x + attn@wo + bo) * gamma + beta.

Sharding: 8 cores = 4 batches x 2 query-halves (1024 queries each). Each
core computes K/V projections only for its own 1024 keys, then a pair-wise
AllGather ([0,1],[2,3],...) exchanges the halves so each core attends over
the batch's full 2048-key sequence.

All four matmul stages run in fp8(e4m3) DoubleRow mode: operands are packed
as [128, 2, N] pair tiles (two 128-row contraction chunks side by side), so
each matmul contracts 256 elements/instruction at 2 fp8 MACs/cell/cycle
(~1.9x measured over bf16; LDWEIGHTS fully pipelines behind the previous
matmul at 512-wide moving operands). PSUM accumulation stays fp32. e4m3
quantization noise (~3.6%/element) decorrelates over the 1024-2048-long
contractions; measured end-to-end max rel err is ~8e-3 vs the 2e-2 gate.
NOTE: DoubleRow weight APs require the pair-dim byte stride to be a
multiple of 16 - a 1-byte-stride ones vector hangs the PE (hence the
[128, 2, 16] ones tile).

Phase order (hides the two collectives behind compute; the CC engine has
a fixed ~50us init floor, so partner data can never arrive early -- the
schedule keeps the PE busy with own-half work until it does):
  KT proj -> AG-KT -> V proj -> AG-V -> QT proj -> scores over own keys ->
  attnV over own keys (m 0..3) into bf16 partials while the gathers land ->
  scores over partner keys -> denominators -> attnV over partner keys
  (drain adds the bf16 partial) interleaved with out-proj by query half so
  the LayerNorm epilogue of half 0 overlaps half 1's matmuls.

Per-core dataflow (d-on-partitions "transposed" layout throughout):
  QT[d,q]   = wq^T @ xq^T (+bq)      (host ships x^T fp8 for the core's half)
  KT[d,k_own] = wk^T @ xq^T (+bk)    --> AllGather --> KT full
  V[k_own,d]  = x @ wv               --> AllGather --> V full
  ST[k,q]   = KT^T @ QT              (contracted over d in PSUM)
  PT[k,q]   = exp(ST/32 - ln16)      (the /16 keeps PT in [0,11] and AT in
              ~[-45,45], inside e4m3's +-240 range; it cancels in the
              normalization since the denominator uses the same PT)
  AT[d,q]   = V^T @ PT ; denom[q] = ones^T @ PT  (DoubleRow ones vector)
  O[q,e]    = AT^T @ wo, then O/denom[q] + xq_aug (host adds bo+bv@wo,
              ships x+bo_eff as bf16), LayerNorm with the elementwise pass
              alternating between DVE and ACT per query block. gamma==1 /
              beta==0 (always true for this problem) takes a fused
              single-pass path; a general two-pass path is kept otherwise.
"""

import numpy as np
import ml_dtypes

import concourse.bass as bass
import concourse.mybir as mybir
import concourse.tile as tile
from concourse import bacc

F32 = mybir.dt.float32
F8 = mybir.dt.float8e4
BF16 = mybir.dt.bfloat16
DR = mybir.MatmulPerfMode.DoubleRow

B, S, D = 4, 2048, 1024
Q = 1024          # queries (and own keys) per core
SCALE = 1.0 / 32.0
NLOG16 = -2.772588722239781   # -ln(16): PT = exp(s)/16
EPS = 1e-6
NP = 4            # d pair-chunks (256 each)
NDC = 8           # 128-wide chunks per 1024
RG = [[0, 1], [2, 3], [4, 5], [6, 7]]


def _bcast_ap(ap_1d, parts=128):
    """[N] dram AP -> [parts, N] AP with 0-stride partition dim."""
    return bass.AP(
        tensor=ap_1d.tensor, offset=ap_1d.offset, ap=[[0, parts]] + list(ap_1d.ap)
    )


def build_nc(trivial_gb: bool = True):
    nc = bacc.Bacc("TRN2", target_bir_lowering=False, debug=False, num_devices=8)

    xt8_d = nc.dram_tensor("xt8", [D, Q], F8, kind="ExternalInput")
    xq_d = nc.dram_tensor("xq", [Q, D], BF16, kind="ExternalInput")
    wq_d = nc.dram_tensor("wq", [D, D], F8, kind="ExternalInput")
    wk_d = nc.dram_tensor("wk", [D, D], F8, kind="ExternalInput")
    wv_d = nc.dram_tensor("wv", [D, D], F8, kind="ExternalInput")
    wo_d = nc.dram_tensor("wo", [D, D], F8, kind="ExternalInput")
    bqT_d = nc.dram_tensor("bqT", [128, NDC], F32, kind="ExternalInput")
    bkT_d = nc.dram_tensor("bkT", [128, NDC], F32, kind="ExternalInput")
    gamma_d = nc.dram_tensor("gamma", [D], F32, kind="ExternalInput")
    beta_d = nc.dram_tensor("beta", [D], F32, kind="ExternalInput")
    out_d = nc.dram_tensor("out", [Q, D], BF16, kind="ExternalOutput")

    with tile.TileContext(nc) as tc:
        with (
            tc.tile_pool(name="small", bufs=1) as p_small,
            tc.tile_pool(name="dram", bufs=1, space="DRAM") as p_dram,
            tc.tile_pool(name="qtp", bufs=NP) as p_qt,
            tc.tile_pool(name="ktp", bufs=NP) as p_kt,
            tc.tile_pool(name="ktlp", bufs=NP) as p_ktl,
            tc.tile_pool(name="vp", bufs=NDC) as p_v,
            tc.tile_pool(name="ptp", bufs=NDC) as p_pt,
            tc.tile_pool(name="atp", bufs=NP) as p_at,
        ):
            kvin_kt = p_dram.tile([D, Q], F8, name="kvin_kt")
            kvout_kt = p_dram.tile([2 * D, Q], F8, name="kvout_kt")
            kvin_v = p_dram.tile([Q, D], F8, name="kvin_v")
            kvout_v = p_dram.tile([S, D], F8, name="kvout_v")

            # ---- constants / small tiles ----
            bqT = p_small.tile([128, NDC], F32)
            nc.gpsimd.dma_start(out=bqT[:, :], in_=bqT_d[:, :])
            bkT = p_small.tile([128, NDC], F32)
            nc.gpsimd.dma_start(out=bkT[:, :], in_=bkT_d[:, :])
            # [128, 2, 16] so the DoubleRow pair dim strides 16 B (HW requires
            # weight-AP step % 16 == 0); only column 0 is used as the ones vector.
            ones = p_small.tile([128, 2, 16], F8)
            nc.vector.memset(ones[:, :, :], 1.0)
            nl16 = p_small.tile([128, 1], F32)
            nc.vector.memset(nl16[:, :], NLOG16)
            eps_t = p_small.tile([128, 1], F32)
            nc.vector.memset(eps_t[:, :], EPS)
            recip = p_small.tile([128, 8], F32)

            pid = nc.sync.partition_id()
            partner_off = (1 - (pid % 2)) * Q   # partner's row base in gathered buffers
            poff512 = (1 - (pid % 2)) * 512     # same, within a half-gather block

            qtp = [p_qt.tile([128, 2, Q], F8, tag="qt", name=f"qtp{j}") for j in range(NP)]
            ktp = [p_kt.tile([128, 2, Q], F8, tag="kt", name=f"ktp{j}") for j in range(NP)]
            ptp = [p_pt.tile([128, 2, Q], F8, tag="pt", name=f"ptp{m}") for m in range(NDC)]

            with (
                tc.tile_pool(name="wp", bufs=3 * NP) as p_w,
                tc.tile_pool(name="xp", bufs=NP) as p_x,
            ):
                wkp = [p_w.tile([128, 2, D], F8, tag="w", name=f"wkp{j}") for j in range(NP)]
                wqp = [p_w.tile([128, 2, D], F8, tag="w", name=f"wqp{j}") for j in range(NP)]
                wvp = [p_w.tile([128, 2, D], F8, tag="w", name=f"wvp{j}") for j in range(NP)]
                xp = [p_x.tile([128, 2, Q], F8, tag="x", name=f"xp{j}") for j in range(NP)]
                qs = [nc.sync, nc.gpsimd, nc.scalar]
                n = 0
                for j in range(NP):
                    for i in range(2):
                        r = 256 * j + 128 * i
                        qs[n % 3].dma_start(out=xp[j][:, i, :], in_=xt8_d[r:r + 128, :])
                        qs[(n + 1) % 3].dma_start(out=wkp[j][:, i, :], in_=wk_d[r:r + 128, :])
                        n += 2
                for j in range(NP):
                    for i in range(2):
                        r = 256 * j + 128 * i
                        nc.sync.dma_start(out=wqp[j][:, i, :], in_=wq_d[r:r + 128, :])
                        nc.gpsimd.dma_start(out=wvp[j][:, i, :], in_=wv_d[r:r + 128, :])

                with tc.tile_pool(name="psp", bufs=6, space="PSUM") as p_ps_proj:
                    # ---- KT_own[d, k_own] (+bk) -> ktp pair tiles + DRAM bounce ----
                    for do in range(NDC):
                        pss = [p_ps_proj.tile([128, 512], F32, tag="ps", name=f"pskt{do}_{h}") for h in range(2)]
                        for j in range(NP):
                            for kh in range(2):
                                nc.tensor.matmul(
                                    pss[kh][:, :],
                                    wkp[j][:, :, 128 * do:128 * (do + 1)],
                                    xp[j][:, :, 512 * kh:512 * (kh + 1)],
                                    start=(j == 0), stop=(j == NP - 1), perf_mode=DR,
                                )
                        for kh in range(2):
                            nc.vector.tensor_scalar(
                                out=ktp[do // 2][:, do % 2, 512 * kh:512 * (kh + 1)],
                                in0=pss[kh][:, :],
                                scalar1=bkT[:, do:do + 1], scalar2=None,
                                op0=mybir.AluOpType.add,
                            )
                    # Exchange KT in two d-halves. Stage ALL kvin stores first:
                    # putting the half-B stores after the ktlp-A loads would
                    # head-of-line-block them on the sync queue behind the AG-A
                    # completion wait, serializing half B behind half A.
                    ktlp = [p_ktl.tile([128, 2, Q], F8, tag="ktl", name=f"ktlp{j}") for j in range(NP)]
                    for do in range(NDC):
                        nc.sync.dma_start(
                            out=kvin_kt[128 * do:128 * (do + 1), :],
                            in_=ktp[do // 2][:, do % 2, :],
                        )
                    nc.gpsimd.collective_compute(
                        "AllGather", mybir.AluOpType.bypass, replica_groups=RG,
                        ins=[kvin_kt[:, :].opt()], outs=[kvout_kt[:, :].opt()],
                    )
                    # partner KT via runtime-parity offset
                    for j in range(NP):
                        for i in range(2):
                            nc.sync.dma_start(
                                out=ktlp[j][:, i, :],
                                in_=kvout_kt[bass.ds(partner_off + 256 * j + 128 * i, 128), :],
                            )

                    # ---- V_own[k_own, e] -> vp pair tiles (m 0..3) + DRAM bounce ----
                    vp = [p_v.tile([128, 2, D], F8, tag="v", name=f"vp{m}") for m in range(NDC)]
                    for kl in range(NDC):
                        pss = [p_ps_proj.tile([128, 512], F32, tag="ps", name=f"psv{kl}_{h}") for h in range(2)]
                        for j in range(NP):
                            for dh in range(2):
                                nc.tensor.matmul(
                                    pss[dh][:, :],
                                    xp[j][:, :, 128 * kl:128 * (kl + 1)],
                                    wvp[j][:, :, 512 * dh:512 * (dh + 1)],
                                    start=(j == 0), stop=(j == NP - 1), perf_mode=DR,
                                )
                        for dh in range(2):
                            nc.scalar.activation(
                                out=vp[kl // 2][:, kl % 2, 512 * dh:512 * (dh + 1)],
                                in_=pss[dh][:, :],
                                func=mybir.ActivationFunctionType.Copy,
                                bias=0.0, scale=1.0,
                            )
                    for kl in range(NDC):
                        nc.gpsimd.dma_start(
                            out=kvin_v[128 * kl:128 * (kl + 1), :],
                            in_=vp[kl // 2][:, kl % 2, :],
                        )
                    nc.gpsimd.collective_compute(
                        "AllGather", mybir.AluOpType.bypass, replica_groups=RG,
                        ins=[kvin_v[:, :].opt()], outs=[kvout_v[:, :].opt()],
                    )
                    # partner V (local key chunks 8..15)
                    for m in range(4, NDC):
                        for i in range(2):
                            nc.sync.dma_start(
                                out=vp[m][:, i, :],
                                in_=kvout_v[bass.ds(partner_off + 256 * (m - 4) + 128 * i, 128), :],
                            )

                    # ---- QT[d,q] (+bq) ----
                    for do in range(NDC):
                        pss = [p_ps_proj.tile([128, 512], F32, tag="ps", name=f"psqt{do}_{h}") for h in range(2)]
                        for j in range(NP):
                            for qh in range(2):
                                nc.tensor.matmul(
                                    pss[qh][:, :],
                                    wqp[j][:, :, 128 * do:128 * (do + 1)],
                                    xp[j][:, :, 512 * qh:512 * (qh + 1)],
                                    start=(j == 0), stop=(j == NP - 1), perf_mode=DR,
                                )
                        for qh in range(2):
                            nc.vector.tensor_scalar(
                                out=qtp[do // 2][:, do % 2, 512 * qh:512 * (qh + 1)],
                                in0=pss[qh][:, :],
                                scalar1=bqT[:, do:do + 1], scalar2=None,
                                op0=mybir.AluOpType.add,
                            )


            with (
                tc.tile_pool(name="ps", bufs=6, space="PSUM") as p_ps,
                tc.tile_pool(name="ps1", bufs=2, space="PSUM") as p_ps1,
            ):
                # bf16 partials for the own-key half of attnV, accumulated while
                # the KT/V gathers are still in flight
                ato = [p_at.tile([128, 2, Q], BF16, tag="ato", name=f"ato{j}") for j in range(NP)]

                # ---- ST -> exp -> PT; local key order: kc 0..7 own, 8..15 partner ----
                for kc in list(range(8)) + ["attnv_own"] + list(range(8, 16)):
                    if kc == "attnv_own":
                        # attnV over OWN keys (m 0..3) -> bf16 partials; fills
                        # the PE while AG-KT/AG-V complete
                        for qh in range(2):
                            for dc in range(NDC):
                                ps = p_ps.tile([128, 512], F32, tag="ps", name=f"psao{qh}_{dc}")
                                for m in range(4):
                                    nc.tensor.matmul(
                                        ps[:, :],
                                        vp[m][:, :, 128 * dc:128 * (dc + 1)],
                                        ptp[m][:, :, 512 * qh:512 * (qh + 1)],
                                        start=(m == 0), stop=(m == 3), perf_mode=DR,
                                    )
                                nc.vector.tensor_copy(
                                    ato[dc // 2][:, dc % 2, 512 * qh:512 * (qh + 1)], ps[:, :]
                                )
                        continue
                    kt_j = ktp if kc < NDC else ktlp
                    kcl = kc % NDC
                    pss = [p_ps.tile([128, 512], F32, tag="ps", name=f"psst{kc}_{h}") for h in range(2)]
                    for j in range(NP):
                        for qh in range(2):
                            nc.tensor.matmul(
                                pss[qh][:, :],
                                kt_j[j][:, :, 128 * kcl:128 * (kcl + 1)],
                                qtp[j][:, :, 512 * qh:512 * (qh + 1)],
                                start=(j == 0), stop=(j == NP - 1), perf_mode=DR,
                            )
                    for qh in range(2):
                        nc.scalar.activation(
                            out=ptp[kc // 2][:, kc % 2, 512 * qh:512 * (qh + 1)],
                            in_=pss[qh][:, :],
                            func=mybir.ActivationFunctionType.Exp,
                            bias=nl16[:, :], scale=SCALE,
                        )

                # ---- denominators: denom[q] = ones^T @ PT ----
                for qp in range(8):
                    ps1 = p_ps1.tile([128, 1], F32, tag="ps1", name=f"ps1_{qp}")
                    for m in range(NDC):
                        nc.tensor.matmul(
                            ps1[:, :],
                            ptp[m][:, :, 128 * qp:128 * (qp + 1)],
                            ones[:, :, 0:1],
                            start=(m == 0), stop=(m == NDC - 1), perf_mode=DR,
                        )
                    nc.vector.reciprocal(recip[:, qp:qp + 1], ps1[:, :])

                # ---- AT[d,q] = V^T @ PT by query-half, with the output
                # projection + LayerNorm for that half interleaved so the
                # elementwise epilogue overlaps the other half's matmuls ----
                atp = [p_at.tile([128, 2, Q], F8, tag="at", name=f"atp{j}") for j in range(NP)]
                with (
                    tc.tile_pool(name="wop", bufs=NP) as p_wo,
                    tc.tile_pool(name="xqp", bufs=4) as p_xq,
                    tc.tile_pool(name="vout", bufs=6) as p_vo,
                    tc.tile_pool(name="lnst", bufs=8) as p_ln,
                ):
                    wop = [p_wo.tile([128, 2, D], F8, tag="wo", name=f"wop{j}") for j in range(NP)]
                    for j in range(NP):
                        for i in range(2):
                            r = 256 * j + 128 * i
                            nc.scalar.dma_start(out=wop[j][:, i, :], in_=wo_d[r:r + 128, :])
                    gam = p_small.tile([128, D], F32)
                    bet = p_small.tile([128, D], F32)
                    if not trivial_gb:
                        nc.gpsimd.dma_start(out=gam[:, :], in_=_bcast_ap(gamma_d[:]))
                        nc.gpsimd.dma_start(out=bet[:, :], in_=_bcast_ap(beta_d[:]))

                    for qh in range(2):
                        for dc in range(NDC):
                            ps = p_ps.tile([128, 512], F32, tag="ps", name=f"psat{qh}_{dc}")
                            for m in range(4, NDC):
                                nc.tensor.matmul(
                                    ps[:, :],
                                    vp[m][:, :, 128 * dc:128 * (dc + 1)],
                                    ptp[m][:, :, 512 * qh:512 * (qh + 1)],
                                    start=(m == 4), stop=(m == NDC - 1), perf_mode=DR,
                                )
                            # AT = partner partial (psum) + own partial (bf16)
                            nc.vector.tensor_add(
                                atp[dc // 2][:, dc % 2, 512 * qh:512 * (qh + 1)],
                                ps[:, :],
                                ato[dc // 2][:, dc % 2, 512 * qh:512 * (qh + 1)],
                            )

                        for qp in range(4 * qh, 4 * qh + 4):
                            v = p_vo.tile([128, D], BF16, tag="v", name=f"v{qp}")
                            sqs = p_vo.tile([128, D], BF16, tag="sqs", name=f"sqs{qp}")
                            xqt_ = p_xq.tile([128, D], BF16, tag="xq", name=f"xqt{qp}")
                            nc.scalar.dma_start(
                                out=xqt_[:, :], in_=xq_d[128 * qp:128 * (qp + 1), :]
                            )
                            st = p_ln.tile([128, 8], F32, tag="st", name=f"st{qp}")
                            for eh in range(2):
                                ps = p_ps.tile([128, 512], F32, tag="ps", name=f"pso{qp}_{eh}")
                                for j in range(NP):
                                    nc.tensor.matmul(
                                        ps[:, :],
                                        atp[j][:, :, 128 * qp:128 * (qp + 1)],
                                        wop[j][:, :, 512 * eh:512 * (eh + 1)],
                                        start=(j == 0), stop=(j == NP - 1), perf_mode=DR,
                                    )
                                # v_half = O/denom + xq_aug; accum = row-sum
                                nc.vector.scalar_tensor_tensor(
                                    out=v[:, 512 * eh:512 * (eh + 1)], in0=ps[:, :],
                                    scalar=recip[:, qp:qp + 1],
                                    in1=xqt_[:, 512 * eh:512 * (eh + 1)],
                                    op0=mybir.AluOpType.mult, op1=mybir.AluOpType.add,
                                    accum_out=st[:, eh:eh + 1],
                                )
                            # E[v^2] via ACT Square + free accum; then mean/var/rstd
                            nc.scalar.activation(
                                out=sqs[:, :], in_=v[:, :],
                                func=mybir.ActivationFunctionType.Square,
                                accum_out=st[:, 2:3],
                            )
                            # mean = (s0+s1)/D ; var = sq/D - mean^2
                            nc.vector.tensor_scalar(
                                out=st[:, 0:1], in0=st[:, 0:1],
                                scalar1=st[:, 1:2], scalar2=1.0 / D,
                                op0=mybir.AluOpType.add, op1=mybir.AluOpType.mult,
                            )
                            nc.vector.tensor_mul(st[:, 1:2], st[:, 0:1], st[:, 0:1])
                            nc.vector.tensor_scalar(
                                out=st[:, 2:3], in0=st[:, 2:3],
                                scalar1=1.0 / D, scalar2=st[:, 1:2],
                                op0=mybir.AluOpType.mult, op1=mybir.AluOpType.subtract,
                            )
                            nc.scalar.activation(
                                out=st[:, 2:3], in_=st[:, 2:3],
                                func=mybir.ActivationFunctionType.Sqrt,
                                bias=eps_t[:, :],
                            )
                            nc.vector.reciprocal(st[:, 2:3], st[:, 2:3])       # rstd
                            vo = p_vo.tile([128, D], BF16, tag="vo", name=f"vo{qp}")
                            if trivial_gb:
                                # gamma==1, beta==0: out = (v - mean)*rstd in one
                                # pass, alternating DVE / ACT per block.
                                if qp % 2 == 0:
                                    nc.vector.tensor_scalar(
                                        out=vo[:, :], in0=v[:, :],
                                        scalar1=st[:, 0:1], scalar2=st[:, 2:3],
                                        op0=mybir.AluOpType.subtract, op1=mybir.AluOpType.mult,
                                    )
                                else:
                                    # ACT: out = rstd*v + (-mean*rstd)
                                    nc.vector.tensor_scalar(
                                        out=st[:, 3:4], in0=st[:, 0:1],
                                        scalar1=st[:, 2:3], scalar2=-1.0,
                                        op0=mybir.AluOpType.mult, op1=mybir.AluOpType.mult,
                                    )
                                    nc.scalar.activation(
                                        out=vo[:, :], in_=v[:, :],
                                        func=mybir.ActivationFunctionType.Identity,
                                        bias=st[:, 3:4], scale=st[:, 2:3],
                                    )
                            else:
                                # out = ((v - mean)*gamma)*rstd + beta  (2 fused DVE ops)
                                nc.vector.scalar_tensor_tensor(
                                    out=vo[:, :], in0=v[:, :], scalar=st[:, 0:1],
                                    in1=gam[:, :],
                                    op0=mybir.AluOpType.subtract, op1=mybir.AluOpType.mult,
                                )
                                nc.vector.scalar_tensor_tensor(
                                    out=vo[:, :], in0=vo[:, :], scalar=st[:, 2:3],
                                    in1=bet[:, :],
                                    op0=mybir.AluOpType.mult, op1=mybir.AluOpType.add,
                                )
                            nc.sync.dma_start(out=out_d[128 * qp:128 * (qp + 1), :], in_=vo[:, :])
    nc.compile()
    return nc


_NC_CACHE = {}


def make_in_maps(inputs):
    x = np.asarray(inputs["inputs"], np.float32)
    wo = np.asarray(inputs["wo"], np.float32)
    f8 = lambda a: np.clip(np.ascontiguousarray(a), -240, 240).astype(ml_dtypes.float8_e4m3)
    bo_eff = np.asarray(inputs["bo"], np.float32) + np.asarray(inputs["bv"], np.float32) @ wo
    shared = {
        "wq": f8(inputs["wq"]), "wk": f8(inputs["wk"]),
        "wv": f8(inputs["wv"]), "wo": f8(wo),
        "bqT": np.ascontiguousarray(np.asarray(inputs["bq"], np.float32).reshape(NDC, 128).T),
        "bkT": np.ascontiguousarray(np.asarray(inputs["bk"], np.float32).reshape(NDC, 128).T),
        "gamma": np.asarray(inputs["gamma"], np.float32),
        "beta": np.asarray(inputs["beta"], np.float32),
    }
    in_maps = []
    for c in range(8):
        b, qh = c // 2, c % 2
        xslab = x[b, Q * qh:Q * (qh + 1), :]
        in_maps.append({
            **shared,
            "xt8": f8(xslab.T),
            "xq": (np.ascontiguousarray(xslab) + bo_eff[None, :]).astype(ml_dtypes.bfloat16),
        })
    return in_maps


def kernel(**inputs) -> np.ndarray:
    from concourse.bass_utils import run_bass_kernel_spmd

    trivial = bool(
        np.all(np.asarray(inputs["gamma"], np.float32) == 1.0)
        and np.all(np.asarray(inputs["beta"], np.float32) == 0.0)
    )
    if trivial not in _NC_CACHE:
        _NC_CACHE[trivial] = build_nc(trivial_gb=trivial)
    res = run_bass_kernel_spmd(_NC_CACHE[trivial], make_in_maps(inputs), core_ids=list(range(8)))
    out = np.empty((B, S, D), np.float32)
    for c in range(8):
        b, qh = c // 2, c % 2
        out[b, Q * qh:Q * (qh + 1), :] = res.results[c]["out"].astype(np.float32)
    return out
